# revision 1
# baseline (speedup 1.0000x reference)
"""Trainium2 Bass kernel for CombinedLoss (focal + dice + boundary-weighted BCE).

Contract: kernel(inputs, targets) takes FULL (64,1,512,512) fp32 arrays and
returns the full scalar loss (fp32). Data-parallel over batch: 8 images per
NeuronCore on 8 cores; host combines per-core partial sums in float64.

Design (engine-balanced, transposed layout):
  Layout: [128 partitions = rows within a 128-row band, free = 32 slots
  (img*4+band) x 512 cols], processed in 4 groups of 8 slots (2 images).
  z  = (2t-1)*x, zh=(t-0.5)*x bf16          (DVE scalar_tensor_tensor)
  q  = exp(-2 zh); bce = ln(1+q)+acc; pt = exp(-bce)+acc; m = copy(t)+acc
                                             (ScalarE, one ACT table)
  Morphology: 2-iter erode/dilate == thresholds of W = conv2(m, 13-pt
  diamond kernel cross(x)cross). Vertical conv -> TensorE banded-stationary
  matmuls (B5/B3/B1 bands over partitions), horizontal shifts -> shifted
  moving-operand views; W accumulates in PSUM per band tile.
  Rows 0,1,126,127 at the 3 internal band boundaries of each image get
  wrong W from band truncation; a separate aux pass (ctx rows reloaded from
  HBM, block-diag stationaries S5/S3/S1) computes W_true for those 96 rows
  and the main per-tile ops exclude them via partition ranges.
  Custom DVE ops (runtime-registered):
    FOCAL: (1-pt)^2*bce, accum     BOUND: (min(W,1)-relu(W-24))*bce, accum
  dice sum(t*pt): v = m*pt (DVE TT 2x) + ones-stationary matmul reduce.
"""

import numpy as np
import operator

N_CORES = 8
IMG = 8            # images per core
H = 512
W = 512
BANDS = 4          # 128-row bands per image
P = 128
SLOTS = IMG * BANDS   # 32, slot = img*4 + band
GROUPS = 4
GS = SLOTS // GROUPS  # 8 slots per group = 2 images
N_TOTAL = 64 * H * W

_CACHE = {}


def _register_dve_op(name, spec):
    from concourse import dve_ops
    from concourse.dve_uop import DveOpSpec
    from concourse.dve_spec import lower
    for op in dve_ops.OPS:
        if op.name == name:
            return op
    opcode = max(dve_ops._SUB_OPCODE_FOR_NAME.values()) + 1
    assert opcode < 0x20
    dve_ops._SUB_OPCODE_FOR_NAME[name] = opcode
    uops = lower(spec, ver="v3")
    sha = DveOpSpec(name=name, opcode=opcode, uops=uops,
                    rd1_en=dve_ops.has_src1(spec)).sha("v3")
    op = dve_ops.DveOp(name, spec, subdim=False, uops_sha={"v3": sha})
    dve_ops.OPS.append(op)
    return op


def _stationaries():
    """Banded vertical-conv stationaries (bf16 host arrays)."""
    import ml_dtypes
    bf = ml_dtypes.bfloat16
    kv0 = [1.0, 2.0, 5.0, 2.0, 1.0]   # dc=0 column of the diamond kernel
    kv1 = [2.0, 2.0, 2.0]             # dc=+-1
    B5 = np.zeros((P, P), np.float32)
    B3 = np.zeros((P, P), np.float32)
    for p in range(P):
        for i in range(max(0, p - 2), min(P, p + 3)):
            B5[p, i] = kv0[p - i + 2]
        for i in range(max(0, p - 1), min(P, p + 2)):
            B3[p, i] = kv1[p - i + 1]
    B1 = np.eye(P, dtype=np.float32)
    # aux block-diag: q=(li,b,k ctx row 0..7) -> j=(li,b,w wrong row 0..3)
    # ctx row k = img row 124+128b+k ; wrong row w = img row 126+128b+w
    # vertical delta = k - w - 2
    S5 = np.zeros((96, 48), np.float32)
    S3 = np.zeros((96, 48), np.float32)
    S1 = np.zeros((96, 48), np.float32)
    # truncated variants: only same-side-of-boundary taps (what the main
    # banded matmuls actually computed at those rows)
    S5t = np.zeros((96, 48), np.float32)
    S3t = np.zeros((96, 48), np.float32)
    S1t = np.zeros((96, 48), np.float32)
    for li in range(4):
        for b in range(3):
            for k in range(8):
                for w in range(4):
                    d = k - w - 2
                    q = li * 24 + b * 8 + k
                    j = li * 12 + b * 4 + w
                    same = (w < 2 and k < 4) or (w >= 2 and k >= 4)
                    if -2 <= d <= 2:
                        S5[q, j] = kv0[d + 2]
                        if same:
                            S5t[q, j] = kv0[d + 2]
                    if -1 <= d <= 1:
                        S3[q, j] = kv1[d + 1]
                        if same:
                            S3t[q, j] = kv1[d + 1]
                    if d == 0:
                        S1[q, j] = 1.0
                        if same:
                            S1t[q, j] = 1.0
    ones = np.ones((P, 1), np.float32)
    return {k: v.astype(bf) for k, v in
            dict(b5=B5, b3=B3, b1=B1, s5=S5, s3=S3, s1=S1,
                 s5t=S5t, s3t=S3t, s1t=S1t, ones=ones).items()}


def _build():
    from concourse import bacc, mybir, tile
    from concourse.dve_spec import (Spec, Src0, Src1, C0, One, Zero,
                                    minn, maxx, sq)

    f32 = mybir.dt.float32
    bf16 = mybir.dt.bfloat16
    Alu = mybir.AluOpType
    Act = mybir.ActivationFunctionType

    FOCAL = _register_dve_op("ANT_FOCAL_SSQ", Spec(
        body=sq(One - Src0) * Src1, accum=operator.add))
    BOUND = _register_dve_op("ANT_BOUND_WDF", Spec(
        body=(minn(Src0, One) - maxx(Src0 - C0, Zero)) * Src1,
        accum=operator.add))
    NBOUND = _register_dve_op("ANT_BOUND_NEG", Spec(
        body=(maxx(Src0 - C0, Zero) - minn(Src0, One)) * Src1,
        accum=operator.add))

    nc = bacc.Bacc("TRN2", target_bir_lowering=False, debug=False,
                   num_devices=N_CORES)

    x_d = nc.dram_tensor("x", [P, SLOTS, W], f32, kind="ExternalInput").ap()
    t_d = nc.dram_tensor("t", [P, SLOTS, W], f32, kind="ExternalInput").ap()
    tctx_d = [nc.dram_tensor(f"tctx{h}", [96, W], f32,
                             kind="ExternalInput").ap() for h in range(2)]
    xwr_d = [nc.dram_tensor(f"xwr{h}", [48, W], f32,
                            kind="ExternalInput").ap() for h in range(2)]
    twr_d = [nc.dram_tensor(f"twr{h}", [48, W], f32,
                            kind="ExternalInput").ap() for h in range(2)]
    b5_d = nc.dram_tensor("b5", [P, P], bf16, kind="ExternalInput").ap()
    b3_d = nc.dram_tensor("b3", [P, P], bf16, kind="ExternalInput").ap()
    b1_d = nc.dram_tensor("b1", [P, P], bf16, kind="ExternalInput").ap()
    s5_d = nc.dram_tensor("s5", [96, 48], bf16, kind="ExternalInput").ap()
    s3_d = nc.dram_tensor("s3", [96, 48], bf16, kind="ExternalInput").ap()
    s1_d = nc.dram_tensor("s1", [96, 48], bf16, kind="ExternalInput").ap()
    s5t_d = nc.dram_tensor("s5t", [96, 48], bf16, kind="ExternalInput").ap()
    s3t_d = nc.dram_tensor("s3t", [96, 48], bf16, kind="ExternalInput").ap()
    s1t_d = nc.dram_tensor("s1t", [96, 48], bf16, kind="ExternalInput").ap()
    ones_d = nc.dram_tensor("ones", [P, 1], bf16, kind="ExternalInput").ap()

    # acc cols: 0:4 sum(bce) per group, 4:8 sum(pt), 8:12 sum(t),
    # 12:16 focal sum
    acc_d = nc.dram_tensor("acc", [P, 16], f32, kind="ExternalOutput").ap()
    accb_d = nc.dram_tensor("accb", [P, SLOTS], f32,
                            kind="ExternalOutput").ap()
    acca_d = nc.dram_tensor("acca", [48, 4], f32, kind="ExternalOutput").ap()
    dice_d = nc.dram_tensor("dice", [1, W], f32, kind="ExternalOutput").ap()

    with tile.TileContext(nc) as tc:
        with (
            tc.tile_pool(name="io", bufs=2) as io,
            tc.tile_pool(name="cn", bufs=1) as cn,
            tc.tile_pool(name="ew", bufs=2) as ew,
            tc.tile_pool(name="ax", bufs=1) as ax,
            tc.tile_pool(name="psw", bufs=1, space="PSUM") as psw,
            tc.tile_pool(name="psd", bufs=1, space="PSUM") as psd,
            tc.tile_pool(name="psa", bufs=1, space="PSUM") as psa,
        ):
            b5 = cn.tile([P, P], bf16, tag="b5")
            b3 = cn.tile([P, P], bf16, tag="b3")
            b1 = cn.tile([P, P], bf16, tag="b1")
            s5 = cn.tile([96, 48], bf16, tag="s5")
            s3 = cn.tile([96, 48], bf16, tag="s3")
            s1 = cn.tile([96, 48], bf16, tag="s1")
            s5t = cn.tile([96, 48], bf16, tag="s5t")
            s3t = cn.tile([96, 48], bf16, tag="s3t")
            s1t = cn.tile([96, 48], bf16, tag="s1t")
            ones = cn.tile([P, 1], bf16, tag="ones")
            for tl, dd in ((b5, b5_d), (b3, b3_d), (b1, b1_d), (s5, s5_d),
                           (s3, s3_d), (s1, s1_d), (s5t, s5t_d),
                           (s3t, s3t_d), (s1t, s1t_d), (ones, ones_d)):
                nc.sync.dma_start(tl[:], dd[:])

            acc = cn.tile([P, 16], f32, tag="acc")
            accb = cn.tile([P, SLOTS], f32, tag="accb")
            acca = cn.tile([48, 4], f32, tag="acca")
            nc.vector.memset(acc[:], 0.0)
            nc.vector.memset(accb[:], 0.0)
            nc.vector.memset(acca[:], 0.0)

            dice_ps = psd.tile([1, W], f32, tag="dice")

            state = {}

            def produce(g):
                xs = io.tile([P, GS, W], f32, tag="xs")
                ts = io.tile([P, GS, W], f32, tag="ts")
                half = GS // 2
                nc.sync.dma_start(xs[:, 0:half, :],
                                  x_d[:, g * GS:g * GS + half, :])
                nc.sync.dma_start(xs[:, half:GS, :],
                                  x_d[:, g * GS + half:(g + 1) * GS, :])
                nc.sync.dma_start(ts[:, 0:half, :],
                                  t_d[:, g * GS:g * GS + half, :])
                nc.sync.dma_start(ts[:, half:GS, :],
                                  t_d[:, g * GS + half:(g + 1) * GS, :])

                zh = ew.tile([P, GS, W], bf16, tag="zh")
                nc.vector.scalar_tensor_tensor(
                    out=zh[:], in0=ts[:], scalar=0.5, in1=xs[:],
                    op0=Alu.subtract, op1=Alu.mult)

                m = ew.tile([P, GS, W + 4], bf16, tag="m")
                nc.vector.memset(m[:, :, 0:2], 0.0)
                nc.vector.memset(m[:, :, W + 2:W + 4], 0.0)
                nc.scalar.activation(m[:, :, 2:2 + W], ts[:], Act.Copy,
                                     accum_out=acc[:, 8 + g:9 + g])
                q = ew.tile([P, GS, W], bf16, tag="q")
                bce = ew.tile([P, GS, W], bf16, tag="bce")
                pt = ew.tile([P, GS, W], bf16, tag="pt")
                nc.scalar.activation(q[:], zh[:], Act.Exp, scale=-2.0)
                nc.scalar.activation(bce[:], q[:], Act.Ln, bias=1.0,
                                     accum_out=acc[:, g:g + 1])
                nc.scalar.activation(pt[:], bce[:], Act.Exp, scale=-1.0,
                                     accum_out=acc[:, 4 + g:5 + g])

                # morphology W per slot: banded matmuls, batched by weight
                # over sub-batches of 4 slots (4 rotating PSUM banks);
                # each W consumed immediately by the BOUND custom op
                junk = ew.tile([P, GS, W], bf16, tag="junk", bufs=1)
                for s0 in (0, 4):
                    sb = [psw.tile([P, W], f32, tag=f"w{i}",
                                   name=f"W_g{g}_s{s0 + i}")
                          for i in range(4)]
                    for i, wt in enumerate(sb):
                        ms = m[:, s0 + i, :]
                        nc.tensor.matmul(wt[:], b5[:], ms[:, 2:2 + W],
                                         start=True, stop=False)
                    for i, wt in enumerate(sb):
                        ms = m[:, s0 + i, :]
                        nc.tensor.matmul(wt[:], b3[:], ms[:, 1:1 + W],
                                         start=False, stop=False)
                        nc.tensor.matmul(wt[:], b3[:], ms[:, 3:3 + W],
                                         start=False, stop=False)
                    for i, wt in enumerate(sb):
                        ms = m[:, s0 + i, :]
                        nc.tensor.matmul(wt[:], b1[:], ms[:, 0:W],
                                         start=False, stop=False)
                        nc.tensor.matmul(wt[:], b1[:], ms[:, 4:4 + W],
                                         start=False, stop=True)
                    # boundary-weighted sums over full tiles; the wrong
                    # contributions at band-boundary rows are cancelled by
                    # the aux NBOUND pass on the truncated "fake" W
                    for i, wt in enumerate(sb):
                        s = s0 + i
                        col = g * GS + s
                        nc.vector._custom_dve(
                            BOUND, out=junk[:, s, :],
                            in0=wt[:], in1=bce[:, s, :],
                            s0=24.0, accum_out=accb[:, col:col + 1])
                state[g] = (m, bce, pt)

            def consume(g):
                m, bce, pt = state.pop(g)
                junk = ew.tile([P, GS, W], bf16, tag="junk2", bufs=1)
                # focal
                nc.vector._custom_dve(
                    FOCAL, out=junk[:], in0=pt[:], in1=bce[:],
                    accum_out=acc[:, 12 + g:13 + g])
                # dice product + ones-matmul reduce
                v = ew.tile([P, GS, W], bf16, tag="v")
                nc.vector.tensor_tensor(out=v[:], in0=m[:, :, 2:2 + W],
                                        in1=pt[:], op=Alu.mult)
                for s in range(GS):
                    nc.tensor.matmul(dice_ps[:], ones[:], v[:, s, :],
                                     start=(g == 0 and s == 0),
                                     stop=(g == GROUPS - 1 and s == GS - 1))

            def aux():
                for h in range(2):
                    tctx = ax.tile([96, W], f32, tag="tctx")
                    xwr = ax.tile([48, W], f32, tag="xwr")
                    twr = ax.tile([48, W], f32, tag="twr")
                    nc.sync.dma_start(tctx[:], tctx_d[h][:])
                    nc.sync.dma_start(xwr[:], xwr_d[h][:])
                    nc.sync.dma_start(twr[:], twr_d[h][:])
                    mctx = ax.tile([96, W + 4], bf16, tag="mctx")
                    nc.vector.memset(mctx[:, 0:2], 0.0)
                    nc.vector.memset(mctx[:, W + 2:W + 4], 0.0)
                    nc.scalar.activation(mctx[:, 2:2 + W], tctx[:], Act.Copy)
                    zw = ax.tile([48, W], bf16, tag="zw")
                    nc.vector.scalar_tensor_tensor(
                        out=zw[:], in0=twr[:], scalar=0.5, in1=xwr[:],
                        op0=Alu.subtract, op1=Alu.mult)
                    qw = ax.tile([48, W], bf16, tag="qw")
                    bw = ax.tile([48, W], bf16, tag="bw")
                    nc.scalar.activation(qw[:], zw[:], Act.Exp, scale=-2.0)
                    nc.scalar.activation(bw[:], qw[:], Act.Ln, bias=1.0)
                    jw = ax.tile([48, W], bf16, tag="jw")
                    for op, col, mats in (
                        (BOUND, h, (s5, s3, s1)),
                        (NBOUND, 2 + h, (s5t, s3t, s1t)),
                    ):
                        wt = psa.tile([48, W], f32, tag="wtrue")
                        nc.tensor.matmul(wt[:], mats[0][:], mctx[:, 2:2 + W],
                                         start=True, stop=False)
                        nc.tensor.matmul(wt[:], mats[1][:], mctx[:, 1:1 + W],
                                         start=False, stop=False)
                        nc.tensor.matmul(wt[:], mats[1][:], mctx[:, 3:3 + W],
                                         start=False, stop=False)
                        nc.tensor.matmul(wt[:], mats[2][:], mctx[:, 0:W],
                                         start=False, stop=False)
                        nc.tensor.matmul(wt[:], mats[2][:], mctx[:, 4:4 + W],
                                         start=False, stop=True)
                        nc.vector._custom_dve(
                            op, out=jw[:], in0=wt[:], in1=bw[:], s0=24.0,
                            accum_out=acca[:, col:col + 1])

            produce(0)
            for g in range(1, GROUPS):
                produce(g)
                consume(g - 1)
            consume(GROUPS - 1)
            aux()

            nc.sync.dma_start(acc_d[:], acc[:])
            nc.sync.dma_start(accb_d[:], accb[:])
            nc.sync.dma_start(acca_d[:], acca[:])
            dsb = cn.tile([1, W], f32, tag="dsb")
            nc.scalar.copy(dsb[:], dice_ps[:])
            nc.sync.dma_start(dice_d[:], dsb[:])

    nc.compile()
    return nc


def _get_nc():
    if "nc" not in _CACHE:
        _CACHE["nc"] = _build()
    return _CACHE["nc"]


def kernel(inputs: np.ndarray, targets: np.ndarray) -> np.ndarray:
    import os
    from concourse.bass_utils import run_bass_kernel_spmd

    nc = _get_nc()
    st = _stationaries()

    x = np.asarray(inputs, dtype=np.float32).reshape(64, H, W)
    t = np.asarray(targets, dtype=np.float32).reshape(64, H, W)

    in_maps = []
    for c in range(N_CORES):
        xc = x[c * IMG:(c + 1) * IMG]     # [8, 512, 512]
        tc_ = t[c * IMG:(c + 1) * IMG]
        # transposed layout: [128 p=row-in-band, slot=img*4+band, 512]
        xT = np.ascontiguousarray(
            xc.reshape(IMG, BANDS, P, W).transpose(2, 0, 1, 3)
            .reshape(P, SLOTS, W))
        tT = np.ascontiguousarray(
            tc_.reshape(IMG, BANDS, P, W).transpose(2, 0, 1, 3)
            .reshape(P, SLOTS, W))
        im = {"x": xT, "t": tT}
        # aux: ctx rows 124+128b..131+128b, wrong rows 126+128b..129+128b
        for h in range(2):
            imgs = tc_[4 * h:4 * h + 4]
            ximgs = xc[4 * h:4 * h + 4]
            tctx = np.stack([imgs[li, 124 + 128 * b:132 + 128 * b, :]
                             for li in range(4) for b in range(3)])
            im[f"tctx{h}"] = np.ascontiguousarray(
                tctx.reshape(96, W).astype(np.float32))
            twr = np.stack([imgs[li, 126 + 128 * b:130 + 128 * b, :]
                            for li in range(4) for b in range(3)])
            im[f"twr{h}"] = np.ascontiguousarray(
                twr.reshape(48, W).astype(np.float32))
            xwr = np.stack([ximgs[li, 126 + 128 * b:130 + 128 * b, :]
                            for li in range(4) for b in range(3)])
            im[f"xwr{h}"] = np.ascontiguousarray(
                xwr.reshape(48, W).astype(np.float32))
        im.update(st)
        in_maps.append(im)

    trace = bool(os.environ.get("BASS_TRACE_KERNEL"))
    res = run_bass_kernel_spmd(nc, in_maps, core_ids=list(range(N_CORES)),
                               trace=trace)
    _CACHE["exec_time_ns"] = res.exec_time_ns

    s_bce = s_pt = s_t = s_focal = s_bnd = s_tpt = 0.0
    for c in range(N_CORES):
        acc = res.results[c]["acc"].astype(np.float64)
        s_bce += acc[:, 0:4].sum()
        s_pt += acc[:, 4:8].sum()
        s_t += acc[:, 8:12].sum()
        s_focal += acc[:, 12:16].sum()
        s_bnd += res.results[c]["accb"].astype(np.float64).sum()
        s_bnd += res.results[c]["acca"].astype(np.float64).sum()
        s_tpt += res.results[c]["dice"].astype(np.float64).sum()

    n = float(N_TOTAL)
    focal_loss = 0.25 * s_focal / n
    sum_p = n - s_t - s_pt + 2.0 * s_tpt
    dice = (2.0 * s_tpt + 1e-6) / (sum_p + s_t + 1e-6)
    dice_loss = 1.0 - dice
    boundary_loss = (s_bce + 5.0 * s_bnd) / n
    loss = 0.3 * focal_loss + 0.4 * dice_loss + 0.3 * boundary_loss
    return np.float32(loss)



# revision 2
# speedup vs baseline: 1.6169x; 1.6169x over previous
"""Trainium2 Bass kernel for CombinedLoss (focal + dice + boundary-weighted BCE).

Contract: kernel(inputs, targets) takes FULL (64,1,512,512) fp32 arrays and
returns the full scalar loss (fp32). Data-parallel over batch: 8 images per
NeuronCore on 8 cores; host combines per-core partial sums in float64.

Design (engine-balanced, transposed layout [128 p = row-in-band,
32 slots = img*4+band, 512 cols], bf16/fp8 inputs):
  Host sends x (bf16), t2 = 32*(t-0.5) (bf16, exact), and m (t zero-padded
  2 cols each side, fp8e4).
  zh = t2*x (DVE TT, 2x bf16 mode); q = exp(-zh/16); bce = ln(1+q) +acc;
  pt = exp(-bce) +acc  (ScalarE, single pinned exp/ln table set).
  FOCAL custom DVE op: (1-pt)^2*bce, accum.
  Morphology: 2-iter erode/dilate == thresholds of W = conv2(m, 13-pt
  diamond). 5 taps computed as 3 fp8 DoubleRow matmuls per slot (two
  vertical-band stationaries per instruction, shifted moving views of the
  padded m). BOUND custom DVE op on PSUM W: (min(W,1)-relu(W-24))*bce, acc.
  Rows 0,1,126,127 at the 3 internal band boundaries of each image get
  wrong W from band truncation; the aux pass (block-diag stationaries
  S5/S3/S1 true + truncated) cancels it exactly via BOUND/NBOUND on 96 rows.
  dice sum(t*pt): PE diagonal trick: chain of [128,128] matmuls
  stationary=t2-block, moving=pt-block accumulated in one PSUM tile; host
  takes trace. sum(t) is never needed: Sum(p)+Sum(t) = n - Sum(pt)
  + 2*Sum(t*pt) (cancellation), and Sum(t*pt) = (trace + 16*Sum(pt))/32.
"""

import numpy as np
import operator

N_CORES = 8
IMG = 8            # images per core
H = 512
W = 512
BANDS = 4          # 128-row bands per image
P = 128
SLOTS = IMG * BANDS   # 32, slot = img*4 + band
GROUPS = 4
GS = SLOTS // GROUPS  # 8 slots per group = 2 images

_CACHE = {}


def _register_dve_op(name, spec):
    from concourse import dve_ops
    from concourse.dve_uop import DveOpSpec
    from concourse.dve_spec import lower
    for op in dve_ops.OPS:
        if op.name == name:
            return op
    opcode = max(dve_ops._SUB_OPCODE_FOR_NAME.values()) + 1
    assert opcode < 0x20
    dve_ops._SUB_OPCODE_FOR_NAME[name] = opcode
    uops = lower(spec, ver="v3")
    sha = DveOpSpec(name=name, opcode=opcode, uops=uops,
                    rd1_en=dve_ops.has_src1(spec)).sha("v3")
    op = dve_ops.DveOp(name, spec, subdim=False, uops_sha={"v3": sha})
    dve_ops.OPS.append(op)
    return op


def _stationaries():
    """Conv stationaries: fp8 DoubleRow pairs + aux block-diag bf16."""
    import ml_dtypes
    bf = ml_dtypes.bfloat16
    f8 = ml_dtypes.float8_e4m3
    kv0 = [1.0, 2.0, 5.0, 2.0, 1.0]   # dc=0 column of the diamond kernel
    kv1 = [2.0, 2.0, 2.0]             # dc=+-1
    B5 = np.zeros((P, P), np.float32)
    B3 = np.zeros((P, P), np.float32)
    for p in range(P):
        for i in range(max(0, p - 2), min(P, p + 3)):
            B5[p, i] = kv0[p - i + 2]
        for i in range(max(0, p - 1), min(P, p + 2)):
            B3[p, i] = kv1[p - i + 1]
    B1 = np.eye(P, dtype=np.float32)
    Z = np.zeros((P, P), np.float32)
    # DoubleRow pairs [K, 2, M]; member i pairs with moving view i.
    # p1: cols (+1, +2) -> (B3*m_-1, B5*m_0); p2: cols (+0, +3) ->
    # (B1*m_-2, B3*m_+1); p3: cols (+0, +4) -> (0, B1*m_+2).
    p1 = np.stack([B3, B5], axis=1)
    p2 = np.stack([B1, B3], axis=1)
    p3 = np.stack([Z, B1], axis=1)
    # aux block-diag: q=(li,b,k ctx row 0..7) -> j=(li,b,w wrong row 0..3)
    # ctx row k = img row 124+128b+k ; wrong row w = img row 126+128b+w
    # vertical delta = k - w - 2
    S5 = np.zeros((96, 48), np.float32)
    S3 = np.zeros((96, 48), np.float32)
    S1 = np.zeros((96, 48), np.float32)
    S5t = np.zeros((96, 48), np.float32)
    S3t = np.zeros((96, 48), np.float32)
    S1t = np.zeros((96, 48), np.float32)
    for li in range(4):
        for b in range(3):
            for k in range(8):
                for w in range(4):
                    d = k - w - 2
                    q = li * 24 + b * 8 + k
                    j = li * 12 + b * 4 + w
                    same = (w < 2 and k < 4) or (w >= 2 and k >= 4)
                    if -2 <= d <= 2:
                        S5[q, j] = kv0[d + 2]
                        if same:
                            S5t[q, j] = kv0[d + 2]
                    if -1 <= d <= 1:
                        S3[q, j] = kv1[d + 1]
                        if same:
                            S3t[q, j] = kv1[d + 1]
                    if d == 0:
                        S1[q, j] = 1.0
                        if same:
                            S1t[q, j] = 1.0
    out = {k: v.astype(bf) for k, v in
           dict(s5=S5, s3=S3, s1=S1, s5t=S5t, s3t=S3t, s1t=S1t).items()}
    out.update({k: v.astype(f8) for k, v in
                dict(p1=p1, p2=p2, p3=p3).items()})
    return out


def _patch_act_tables():
    """Pin exp/ln/copy activations to the one table set containing all of
    them (natural_log_exp_and_others) so the kernel does a single
    ACT_TABLE_LOAD instead of thrashing between per-function sets."""
    from concourse import bacc as bacc_mod, hw_specs
    orig = hw_specs.get_activation_tables
    keep = "natural_log_exp_and_others"

    def patched(arch):
        t = orig(arch)
        pin = set(t[keep])
        return {k: (v if k == keep else {f for f in v if f not in pin})
                for k, v in t.items()}

    bacc_mod.get_activation_tables = patched
    return lambda: setattr(bacc_mod, "get_activation_tables", orig)


def _build():
    from concourse import bacc, mybir, tile
    from bass_rust import AP
    from concourse.dve_spec import (Spec, Src0, Src1, C0, One, Zero,
                                    minn, maxx, sq)

    f32 = mybir.dt.float32
    bf16 = mybir.dt.bfloat16
    fp8 = mybir.dt.float8e4
    Alu = mybir.AluOpType
    Act = mybir.ActivationFunctionType
    DR = mybir.MatmulPerfMode.DoubleRow

    FOCAL = _register_dve_op("ANT_FOCAL_SSQ", Spec(
        body=sq(One - Src0) * Src1, accum=operator.add))
    BOUND = _register_dve_op("ANT_BOUND_WDF", Spec(
        body=(minn(Src0, One) - maxx(Src0 - C0, Zero)) * Src1,
        accum=operator.add))
    NBOUND = _register_dve_op("ANT_BOUND_NEG", Spec(
        body=(maxx(Src0 - C0, Zero) - minn(Src0, One)) * Src1,
        accum=operator.add))

    unpatch = _patch_act_tables()
    nc = bacc.Bacc("TRN2", target_bir_lowering=False, debug=False,
                   num_devices=N_CORES)

    x_d = nc.dram_tensor("x", [P, SLOTS, W], bf16, kind="ExternalInput").ap()
    t2_d = nc.dram_tensor("t2", [P, SLOTS, W], bf16,
                          kind="ExternalInput").ap()
    m_d = nc.dram_tensor("m", [P, SLOTS, W + 4], fp8,
                         kind="ExternalInput").ap()
    mctx_d = [nc.dram_tensor(f"mctx{h}", [96, W + 4], bf16,
                             kind="ExternalInput").ap() for h in range(2)]
    xwr_d = [nc.dram_tensor(f"xwr{h}", [48, W], bf16,
                            kind="ExternalInput").ap() for h in range(2)]
    twr_d = [nc.dram_tensor(f"twr{h}", [48, W], bf16,
                            kind="ExternalInput").ap() for h in range(2)]
    p1_d = nc.dram_tensor("p1", [P, 2, P], fp8, kind="ExternalInput").ap()
    p2_d = nc.dram_tensor("p2", [P, 2, P], fp8, kind="ExternalInput").ap()
    p3_d = nc.dram_tensor("p3", [P, 2, P], fp8, kind="ExternalInput").ap()
    s5_d = nc.dram_tensor("s5", [96, 48], bf16, kind="ExternalInput").ap()
    s3_d = nc.dram_tensor("s3", [96, 48], bf16, kind="ExternalInput").ap()
    s1_d = nc.dram_tensor("s1", [96, 48], bf16, kind="ExternalInput").ap()
    s5t_d = nc.dram_tensor("s5t", [96, 48], bf16, kind="ExternalInput").ap()
    s3t_d = nc.dram_tensor("s3t", [96, 48], bf16, kind="ExternalInput").ap()
    s1t_d = nc.dram_tensor("s1t", [96, 48], bf16, kind="ExternalInput").ap()

    # acc cols: 0:4 sum(bce) per group, 4:8 sum(pt), 8:12 focal sum
    acc_d = nc.dram_tensor("acc", [P, 12], f32, kind="ExternalOutput").ap()
    accb_d = nc.dram_tensor("accb", [P, 8], f32, kind="ExternalOutput").ap()
    acca_d = nc.dram_tensor("acca", [48, 4], f32, kind="ExternalOutput").ap()
    dice_d = nc.dram_tensor("dice", [P, P], f32, kind="ExternalOutput").ap()

    with tile.TileContext(nc) as tc:
        with (
            tc.tile_pool(name="io", bufs=2) as io,
            tc.tile_pool(name="cn", bufs=1) as cn,
            tc.tile_pool(name="ew", bufs=2) as ew,
            tc.tile_pool(name="jk", bufs=1) as jk,
            tc.tile_pool(name="ax", bufs=1) as ax,
            tc.tile_pool(name="psw", bufs=1, space="PSUM") as psw,
            tc.tile_pool(name="psd", bufs=1, space="PSUM") as psd,
            tc.tile_pool(name="psa", bufs=2, space="PSUM") as psa,
        ):
            p1 = cn.tile([P, 2, P], fp8, tag="p1")
            p2 = cn.tile([P, 2, P], fp8, tag="p2")
            p3 = cn.tile([P, 2, P], fp8, tag="p3")
            s5 = cn.tile([96, 48], bf16, tag="s5")
            s3 = cn.tile([96, 48], bf16, tag="s3")
            s1 = cn.tile([96, 48], bf16, tag="s1")
            s5t = cn.tile([96, 48], bf16, tag="s5t")
            s3t = cn.tile([96, 48], bf16, tag="s3t")
            s1t = cn.tile([96, 48], bf16, tag="s1t")
            for tl, dd in ((p1, p1_d), (p2, p2_d), (p3, p3_d), (s5, s5_d),
                           (s3, s3_d), (s1, s1_d), (s5t, s5t_d),
                           (s3t, s3t_d), (s1t, s1t_d)):
                nc.sync.dma_start(tl[:], dd[:])

            acc = cn.tile([P, 12], f32, tag="acc")
            accb = cn.tile([P, 8], f32, tag="accb")
            acca = cn.tile([48, 4], f32, tag="acca")
            nc.vector.memset(acc[:], 0.0)
            nc.vector.memset(accb[:], 0.0)
            nc.vector.memset(acca[:], 0.0)

            dice_ps = psd.tile([P, P], f32, tag="dice")

            # aux inputs loaded early so the tail compute never waits on DMA
            auxt = []
            for h in range(2):
                mctx = ax.tile([96, W + 4], bf16, tag=f"mctx{h}")
                xw = ax.tile([48, W], bf16, tag=f"xw{h}")
                tw = ax.tile([48, W], bf16, tag=f"tw{h}")
                nc.sync.dma_start(mctx[:], mctx_d[h][:])
                nc.sync.dma_start(xw[:], xwr_d[h][:])
                nc.sync.dma_start(tw[:], twr_d[h][:])
                auxt.append((mctx, xw, tw))

            state = {}

            def conv_rhs(ms, s, c0, step):
                base = ms[:, s, :]
                pd = list(base.ap[0])
                return AP(base.tensor, base.offset + c0,
                          [pd, [step, 2], [1, W]])

            def produce(g):
                xs = io.tile([P, GS, W], bf16, tag="xs")
                ts2 = io.tile([P, GS, W], bf16, tag="ts2")
                ms = io.tile([P, GS, W + 4], fp8, tag="ms")
                half = GS // 2
                nc.sync.dma_start(xs[:, 0:half, :],
                                  x_d[:, g * GS:g * GS + half, :])
                nc.sync.dma_start(xs[:, half:GS, :],
                                  x_d[:, g * GS + half:(g + 1) * GS, :])
                nc.sync.dma_start(ts2[:, 0:half, :],
                                  t2_d[:, g * GS:g * GS + half, :])
                nc.sync.dma_start(ts2[:, half:GS, :],
                                  t2_d[:, g * GS + half:(g + 1) * GS, :])
                nc.sync.dma_start(ms[:], m_d[:, g * GS:(g + 1) * GS, :])

                zh = ew.tile([P, GS, W], bf16, tag="zh")
                nc.vector.tensor_tensor(out=zh[:], in0=ts2[:], in1=xs[:],
                                        op=Alu.mult)
                q = ew.tile([P, GS, W], bf16, tag="q")
                nc.scalar.activation(q[:], zh[:], Act.Exp, scale=-0.0625)
                nb = ew.tile([P, GS, W], bf16, tag="nb")
                nc.scalar.activation(nb[:], q[:], Act.Ln, bias=1.0,
                                     accum_out=acc[:, g:g + 1])
                pt = ew.tile([P, GS, W], bf16, tag="pt")
                nc.scalar.activation(pt[:], nb[:], Act.Exp, scale=-1.0,
                                     accum_out=acc[:, 4 + g:5 + g])

                junk2 = jk.tile([P, GS, W], bf16, tag="junk2")
                nc.vector._custom_dve(
                    FOCAL, out=junk2[:], in0=pt[:], in1=nb[:],
                    accum_out=acc[:, 8 + g:9 + g])

                # morphology W: weight-stationary fp8 DoubleRow batches of
                # 4 slots; W consumed immediately by BOUND
                junk = jk.tile([P, 4, W], bf16, tag="junk")
                for h in range(2):
                    wt = psw.tile([P, 4, W], f32, tag="wt",
                                  name=f"W_g{g}_h{h}")
                    for wi, (pw, c0, st) in enumerate(
                            ((p1, 1, 1), (p2, 0, 3), (p3, 0, 4))):
                        for i in range(4):
                            s = h * 4 + i
                            nc.tensor.matmul(wt[:, i, :], pw[:],
                                             conv_rhs(ms, s, c0, st),
                                             start=(wi == 0),
                                             stop=(wi == 2),
                                             perf_mode=DR)
                    nc.vector._custom_dve(
                        BOUND, out=junk[:],
                        in0=wt[:], in1=nb[:, h * 4:(h + 1) * 4, :],
                        s0=24.0, accum_out=accb[:, 2 * g + h:2 * g + h + 1])
                state[g] = (ts2, pt)

            def consume(g):
                ts2, pt = state.pop(g)
                for s in range(GS):
                    for k in range(4):
                        nc.tensor.matmul(
                            dice_ps[:],
                            ts2[:, s, k * P:(k + 1) * P],
                            pt[:, s, k * P:(k + 1) * P],
                            start=(g == 0 and s == 0 and k == 0),
                            stop=(g == GROUPS - 1 and s == GS - 1
                                  and k == 3))

            def aux():
                for h in range(2):
                    mctx, xw, tw = auxt[h]
                    zw = ax.tile([48, W], bf16, tag="zw")
                    nc.vector.scalar_tensor_tensor(
                        out=zw[:], in0=tw[:], scalar=0.5, in1=xw[:],
                        op0=Alu.subtract, op1=Alu.mult)
                    qw = ax.tile([48, W], bf16, tag="qw")
                    bw = ax.tile([48, W], bf16, tag="bw")
                    nc.scalar.activation(qw[:], zw[:], Act.Exp, scale=-2.0)
                    nc.scalar.activation(bw[:], qw[:], Act.Ln, bias=1.0)
                    jw = ax.tile([48, W], bf16, tag="jw")
                    for op, col, mats in (
                        (BOUND, h, (s5, s3, s1)),
                        (NBOUND, 2 + h, (s5t, s3t, s1t)),
                    ):
                        wt = psa.tile([48, W], f32, tag="wtrue")
                        nc.tensor.matmul(wt[:], mats[0][:], mctx[:, 2:2 + W],
                                         start=True, stop=False)
                        nc.tensor.matmul(wt[:], mats[1][:], mctx[:, 1:1 + W],
                                         start=False, stop=False)
                        nc.tensor.matmul(wt[:], mats[1][:], mctx[:, 3:3 + W],
                                         start=False, stop=False)
                        nc.tensor.matmul(wt[:], mats[2][:], mctx[:, 0:W],
                                         start=False, stop=False)
                        nc.tensor.matmul(wt[:], mats[2][:], mctx[:, 4:4 + W],
                                         start=False, stop=True)
                        nc.vector._custom_dve(
                            op, out=jw[:], in0=wt[:], in1=bw[:], s0=24.0,
                            accum_out=acca[:, col:col + 1])

            produce(0)
            produce(1)
            consume(0)
            produce(2)
            consume(1)
            produce(3)
            aux()
            consume(2)
            consume(3)

            nc.sync.dma_start(acc_d[:], acc[:])
            nc.sync.dma_start(accb_d[:], accb[:])
            nc.sync.dma_start(acca_d[:], acca[:])
            dsb = cn.tile([P, P], f32, tag="dsb")
            nc.scalar.copy(dsb[:], dice_ps[:])
            nc.sync.dma_start(dice_d[:], dsb[:])

    nc.compile()
    unpatch()
    return nc


def _get_nc():
    if "nc" not in _CACHE:
        _CACHE["nc"] = _build()
    return _CACHE["nc"]


def kernel(inputs: np.ndarray, targets: np.ndarray) -> np.ndarray:
    import os
    import ml_dtypes
    from concourse.bass_utils import run_bass_kernel_spmd

    bf = ml_dtypes.bfloat16
    f8 = ml_dtypes.float8_e4m3

    nc = _get_nc()
    st = _stationaries()

    x = np.asarray(inputs, dtype=np.float32).reshape(64, H, W)
    t = np.asarray(targets, dtype=np.float32).reshape(64, H, W)

    in_maps = []
    for c in range(N_CORES):
        xc = x[c * IMG:(c + 1) * IMG]     # [8, 512, 512]
        tc_ = t[c * IMG:(c + 1) * IMG]
        # transposed layout: [128 p=row-in-band, slot=img*4+band, 512]
        xT = np.ascontiguousarray(
            xc.reshape(IMG, BANDS, P, W).transpose(2, 0, 1, 3)
            .reshape(P, SLOTS, W)).astype(bf)
        tT = (tc_.reshape(IMG, BANDS, P, W).transpose(2, 0, 1, 3)
              .reshape(P, SLOTS, W))
        t2 = np.ascontiguousarray(32.0 * (tT - 0.5)).astype(bf)
        mp = np.zeros((P, SLOTS, W + 4), np.float32)
        mp[:, :, 2:2 + W] = tT
        im = {"x": xT, "t2": t2, "m": mp.astype(f8)}
        # aux: ctx rows 124+128b..131+128b, wrong rows 126+128b..129+128b
        for h in range(2):
            imgs = tc_[4 * h:4 * h + 4]
            ximgs = xc[4 * h:4 * h + 4]
            tctx = np.stack([imgs[li, 124 + 128 * b:132 + 128 * b, :]
                             for li in range(4) for b in range(3)])
            mctx = np.zeros((96, W + 4), np.float32)
            mctx[:, 2:2 + W] = tctx.reshape(96, W)
            im[f"mctx{h}"] = mctx.astype(bf)
            twr = np.stack([imgs[li, 126 + 128 * b:130 + 128 * b, :]
                            for li in range(4) for b in range(3)])
            im[f"twr{h}"] = np.ascontiguousarray(
                twr.reshape(48, W)).astype(bf)
            xwr = np.stack([ximgs[li, 126 + 128 * b:130 + 128 * b, :]
                            for li in range(4) for b in range(3)])
            im[f"xwr{h}"] = np.ascontiguousarray(
                xwr.reshape(48, W)).astype(bf)
        im.update(st)
        in_maps.append(im)

    trace = bool(os.environ.get("BASS_TRACE_KERNEL"))
    res = run_bass_kernel_spmd(nc, in_maps, core_ids=list(range(N_CORES)),
                               trace=trace)
    _CACHE["exec_time_ns"] = res.exec_time_ns

    s_bce = s_pt = s_focal = s_bnd = s_diag = 0.0
    for c in range(N_CORES):
        acc = res.results[c]["acc"].astype(np.float64)
        s_bce += acc[:, 0:4].sum()
        s_pt += acc[:, 4:8].sum()
        s_focal += acc[:, 8:12].sum()
        s_bnd += res.results[c]["accb"].astype(np.float64).sum()
        s_bnd += res.results[c]["acca"].astype(np.float64).sum()
        s_diag += np.trace(res.results[c]["dice"].astype(np.float64))

    n = float(64 * H * W)
    s_tpt = (s_diag + 16.0 * s_pt) / 32.0
    focal_loss = 0.25 * s_focal / n
    denom = n - s_pt + 2.0 * s_tpt
    dice = (2.0 * s_tpt + 1e-6) / (denom + 1e-6)
    dice_loss = 1.0 - dice
    boundary_loss = (s_bce + 5.0 * s_bnd) / n
    loss = 0.3 * focal_loss + 0.4 * dice_loss + 0.3 * boundary_loss
    return np.float32(loss)


# revision 17
# speedup vs baseline: 1.6300x; 1.0081x over previous
"""Trainium2 Bass kernel for CombinedLoss (focal + dice + boundary-weighted BCE).

Contract: kernel(inputs, targets) takes FULL (64,1,512,512) fp32 arrays and
returns the full scalar loss (fp32). Data-parallel over batch: 8 images per
NeuronCore on 8 cores; host combines per-core partial sums in float64.

Design (engine-balanced, transposed layout [128 p = row-in-band,
32 slots = img*4+band, 512 cols], bf16/fp8 inputs):
  Host sends x (bf16), t2 = 32*(t-0.5) (bf16, exact), and m (t zero-padded
  2 cols each side, fp8e4).
  zh = t2*x (DVE TT, 2x bf16 mode); q = exp(-zh/16); bce = ln(1+q) +acc;
  pt = exp(-bce) +acc  (ScalarE, single pinned exp/ln table set).
  FOCAL custom DVE op: (1-pt)^2*bce, accum.
  Morphology: 2-iter erode/dilate == thresholds of W = conv2(m, 13-pt
  diamond). 5 taps computed as 3 fp8 DoubleRow matmuls per slot (two
  vertical-band stationaries per instruction, shifted moving views of the
  padded m). BOUND custom DVE op on PSUM W: (min(W,1)-relu(W-24))*bce, acc.
  Rows 0,1,126,127 at the 3 internal band boundaries of each image get
  wrong W from band truncation; the aux pass (block-diag stationaries
  S5/S3/S1 true + truncated) cancels it exactly via BOUND/NBOUND on 96 rows.
  dice sum(t*pt): GpSimd (otherwise idle) scalar_tensor_tensor
  (t2/32)*pt with accum = Sum((t-0.5)*pt); zh also on GpSimd. sum(t) is
  never needed: Sum(p)+Sum(t) = n - Sum(pt) + 2*Sum(t*pt) (cancellation),
  and Sum(t*pt) = Sum((t-0.5)*pt) + 0.5*Sum(pt).
"""

import numpy as np
import operator

N_CORES = 8
IMG = 8            # images per core
H = 512
W = 512
BANDS = 4          # 128-row bands per image
P = 128
SLOTS = IMG * BANDS   # 32, slot = img*4 + band
GROUPS = 4
GS = SLOTS // GROUPS  # 8 slots per group = 2 images

_CACHE = {}


def _register_dve_op(name, spec):
    from concourse import dve_ops
    from concourse.dve_uop import DveOpSpec
    from concourse.dve_spec import lower
    for op in dve_ops.OPS:
        if op.name == name:
            return op
    opcode = max(dve_ops._SUB_OPCODE_FOR_NAME.values()) + 1
    assert opcode < 0x20
    dve_ops._SUB_OPCODE_FOR_NAME[name] = opcode
    uops = lower(spec, ver="v3")
    sha = DveOpSpec(name=name, opcode=opcode, uops=uops,
                    rd1_en=dve_ops.has_src1(spec)).sha("v3")
    op = dve_ops.DveOp(name, spec, subdim=False, uops_sha={"v3": sha})
    dve_ops.OPS.append(op)
    return op


def _stationaries():
    """Conv stationaries: fp8 DoubleRow pairs + aux block-diag bf16."""
    import ml_dtypes
    bf = ml_dtypes.bfloat16
    f8 = ml_dtypes.float8_e4m3
    kv0 = [1.0, 2.0, 5.0, 2.0, 1.0]   # dc=0 column of the diamond kernel
    kv1 = [2.0, 2.0, 2.0]             # dc=+-1
    B5 = np.zeros((P, P), np.float32)
    B3 = np.zeros((P, P), np.float32)
    for p in range(P):
        for i in range(max(0, p - 2), min(P, p + 3)):
            B5[p, i] = kv0[p - i + 2]
        for i in range(max(0, p - 1), min(P, p + 2)):
            B3[p, i] = kv1[p - i + 1]
    B1 = np.eye(P, dtype=np.float32)
    Z = np.zeros((P, P), np.float32)
    # DoubleRow pairs [K, 2, M]; member i pairs with moving view i.
    # p1: cols (+1, +2) -> (B3*m_-1, B5*m_0); p2: cols (+0, +3) ->
    # (B1*m_-2, B3*m_+1); p3: cols (+0, +4) -> (0, B1*m_+2).
    p1 = np.stack([B3, B5], axis=1)
    p2 = np.stack([B1, B3], axis=1)
    p3 = np.stack([Z, B1], axis=1)
    # aux block-diag: q=(li,b,k ctx row 0..7) -> j=(li,b,w wrong row 0..3)
    # ctx row k = img row 124+128b+k ; wrong row w = img row 126+128b+w
    # vertical delta = k - w - 2
    S5 = np.zeros((96, 48), np.float32)
    S3 = np.zeros((96, 48), np.float32)
    S1 = np.zeros((96, 48), np.float32)
    S5t = np.zeros((96, 48), np.float32)
    S3t = np.zeros((96, 48), np.float32)
    S1t = np.zeros((96, 48), np.float32)
    for li in range(4):
        for b in range(3):
            for k in range(8):
                for w in range(4):
                    d = k - w - 2
                    q = li * 24 + b * 8 + k
                    j = li * 12 + b * 4 + w
                    same = (w < 2 and k < 4) or (w >= 2 and k >= 4)
                    if -2 <= d <= 2:
                        S5[q, j] = kv0[d + 2]
                        if same:
                            S5t[q, j] = kv0[d + 2]
                    if -1 <= d <= 1:
                        S3[q, j] = kv1[d + 1]
                        if same:
                            S3t[q, j] = kv1[d + 1]
                    if d == 0:
                        S1[q, j] = 1.0
                        if same:
                            S1t[q, j] = 1.0
    Z48 = np.zeros((96, 48), np.float32)
    out = {k: v.astype(f8) for k, v in dict(
        p1=p1, p2=p2, p3=p3,
        a1=np.stack([S3, S5], axis=1),
        a2=np.stack([S1, S3], axis=1),
        a3=np.stack([Z48, S1], axis=1),
        a1t=np.stack([S3t, S5t], axis=1),
        a2t=np.stack([S1t, S3t], axis=1),
        a3t=np.stack([Z48, S1t], axis=1)).items()}
    del bf
    return out


def _patch_act_tables():
    """Pin exp/ln/copy activations to the one table set containing all of
    them (natural_log_exp_and_others) so the kernel does a single
    ACT_TABLE_LOAD instead of thrashing between per-function sets."""
    from concourse import bacc as bacc_mod, hw_specs
    orig = hw_specs.get_activation_tables
    keep = "natural_log_exp_and_others"

    def patched(arch):
        t = orig(arch)
        pin = set(t[keep])
        return {k: (v if k == keep else {f for f in v if f not in pin})
                for k, v in t.items()}

    bacc_mod.get_activation_tables = patched
    return lambda: setattr(bacc_mod, "get_activation_tables", orig)


def _build():
    from concourse import bacc, mybir, tile
    from bass_rust import AP
    from concourse.dve_spec import (Spec, Src0, Src1, C0, One, Zero,
                                    minn, maxx, sq)

    f32 = mybir.dt.float32
    bf16 = mybir.dt.bfloat16
    fp8 = mybir.dt.float8e4
    Alu = mybir.AluOpType
    Act = mybir.ActivationFunctionType
    DR = mybir.MatmulPerfMode.DoubleRow

    FOCAL = _register_dve_op("ANT_FOCAL_SSQ", Spec(
        body=sq(One - Src0) * Src1, accum=operator.add))
    BOUND = _register_dve_op("ANT_BOUND_WDF", Spec(
        body=(minn(Src0, One) - maxx(Src0 - C0, Zero)) * Src1,
        accum=operator.add))
    NBOUND = _register_dve_op("ANT_BOUND_NEG", Spec(
        body=(maxx(Src0 - C0, Zero) - minn(Src0, One)) * Src1,
        accum=operator.add))

    unpatch = _patch_act_tables()
    nc = bacc.Bacc("TRN2", target_bir_lowering=False, debug=False,
                   num_devices=N_CORES)

    x_d = nc.dram_tensor("x", [P, SLOTS, W], bf16, kind="ExternalInput").ap()
    t2_d = nc.dram_tensor("t2", [P, SLOTS, W], bf16,
                          kind="ExternalInput").ap()
    m_d = nc.dram_tensor("m", [P, SLOTS, W + 4], fp8,
                         kind="ExternalInput").ap()
    mctx_d = [nc.dram_tensor(f"mctx{h}", [96, W + 4], fp8,
                             kind="ExternalInput").ap() for h in range(2)]
    xwr_d = [nc.dram_tensor(f"xwr{h}", [48, W], bf16,
                            kind="ExternalInput").ap() for h in range(2)]
    twr_d = [nc.dram_tensor(f"twr{h}", [48, W], bf16,
                            kind="ExternalInput").ap() for h in range(2)]
    p1_d = nc.dram_tensor("p1", [P, 2, P], fp8, kind="ExternalInput").ap()
    p2_d = nc.dram_tensor("p2", [P, 2, P], fp8, kind="ExternalInput").ap()
    p3_d = nc.dram_tensor("p3", [P, 2, P], fp8, kind="ExternalInput").ap()
    aux_d = {k: nc.dram_tensor(k, [96, 2, 48], fp8, kind="ExternalInput").ap()
             for k in ("a1", "a2", "a3", "a1t", "a2t", "a3t")}

    # acc cols: 0:4 sum(bce) per group, 4:8 sum(pt), 8:12 focal sum,
    # 12:16 sum((t-0.5)*pt)
    acc_d = nc.dram_tensor("acc", [P, 16], f32, kind="ExternalOutput").ap()
    accb_d = nc.dram_tensor("accb", [P, 8], f32, kind="ExternalOutput").ap()
    acca_d = nc.dram_tensor("acca", [48, 4], f32, kind="ExternalOutput").ap()
    dice_d = nc.dram_tensor("dice", [P, P], f32, kind="ExternalOutput").ap()

    with tile.TileContext(nc) as tc:
        with (
            tc.tile_pool(name="io", bufs=2) as io,
            tc.tile_pool(name="cn", bufs=1) as cn,
            tc.tile_pool(name="ew", bufs=2) as ew,
            tc.tile_pool(name="jk", bufs=1) as jk,
            tc.tile_pool(name="ax", bufs=1) as ax,
            tc.tile_pool(name="psw", bufs=1, space="PSUM") as psw,
            tc.tile_pool(name="psd", bufs=1, space="PSUM") as psd,
            tc.tile_pool(name="psa", bufs=2, space="PSUM") as psa,
        ):
            p1 = cn.tile([P, 2, P], fp8, tag="p1")
            p2 = cn.tile([P, 2, P], fp8, tag="p2")
            p3 = cn.tile([P, 2, P], fp8, tag="p3")
            auxw = {}
            for k in ("a1", "a2", "a3", "a1t", "a2t", "a3t"):
                auxw[k] = cn.tile([96, 2, 48], fp8, tag=k, name=k)
                nc.sync.dma_start(auxw[k][:], aux_d[k][:])
            for tl, dd in ((p1, p1_d), (p2, p2_d), (p3, p3_d)):
                nc.sync.dma_start(tl[:], dd[:])

            acc = cn.tile([P, 16], f32, tag="acc")
            accb = cn.tile([P, 8], f32, tag="accb")
            acca = cn.tile([48, 4], f32, tag="acca")
            nc.vector.memset(acc[:], 0.0)
            nc.vector.memset(accb[:], 0.0)
            nc.vector.memset(acca[:], 0.0)

            dice_ps = psd.tile([P, P], f32, tag="dice")
            state = {}

            # aux inputs loaded early so the tail compute never waits on DMA
            auxt = []
            for h in range(2):
                mctx = ax.tile([96, W + 4], fp8, tag=f"mctx{h}")
                xw = ax.tile([48, W], bf16, tag=f"xw{h}")
                tw = ax.tile([48, W], bf16, tag=f"tw{h}")
                nc.sync.dma_start(mctx[:], mctx_d[h][:])
                nc.sync.dma_start(xw[:], xwr_d[h][:])
                nc.sync.dma_start(tw[:], twr_d[h][:])
                auxt.append((mctx, xw, tw))

            def conv_rhs(ms, s, c0, step):
                base = ms[:, s, :]
                pd = list(base.ap[0])
                return AP(base.tensor, base.offset + c0,
                          [pd, [step, 2], [1, W]])

            def produce(g):
                xs = io.tile([P, GS, W], bf16, tag="xs")
                ts2 = io.tile([P, GS, W], bf16, tag="ts2")
                ms = io.tile([P, GS, W + 4], fp8, tag="ms")
                half = GS // 2
                nc.sync.dma_start(xs[:, 0:half, :],
                                  x_d[:, g * GS:g * GS + half, :])
                nc.sync.dma_start(xs[:, half:GS, :],
                                  x_d[:, g * GS + half:(g + 1) * GS, :])
                nc.sync.dma_start(ts2[:, 0:half, :],
                                  t2_d[:, g * GS:g * GS + half, :])
                nc.sync.dma_start(ts2[:, half:GS, :],
                                  t2_d[:, g * GS + half:(g + 1) * GS, :])
                nc.sync.dma_start(ms[:], m_d[:, g * GS:(g + 1) * GS, :])

                zh = ew.tile([P, GS, W], bf16, tag="zh")
                nc.vector.tensor_tensor(out=zh[:], in0=ts2[:], in1=xs[:],
                                        op=Alu.mult)
                q = ew.tile([P, GS, W], bf16, tag="q")
                nc.scalar.activation(q[:], zh[:], Act.Exp, scale=-0.0625)
                nb = ew.tile([P, GS, W], bf16, tag="nb")
                nc.scalar.activation(nb[:], q[:], Act.Ln, bias=1.0,
                                     accum_out=acc[:, g:g + 1])
                pt = ew.tile([P, GS, W], bf16, tag="pt")
                nc.scalar.activation(pt[:], nb[:], Act.Exp, scale=-1.0,
                                     accum_out=acc[:, 4 + g:5 + g])

                junk2 = jk.tile([P, GS, W], bf16, tag="junk2")
                nc.vector._custom_dve(
                    FOCAL, out=junk2[:], in0=pt[:], in1=nb[:],
                    accum_out=acc[:, 8 + g:9 + g])

                # morphology W: weight-stationary fp8 DoubleRow batches of
                # 4 slots; W consumed immediately by BOUND
                junk = jk.tile([P, 4, W], bf16, tag="junk")
                for h in range(2):
                    wt = psw.tile([P, 4, W], f32, tag="wt",
                                  name=f"W_g{g}_h{h}")
                    for wi, (pw, c0, st) in enumerate(
                            ((p1, 1, 1), (p2, 0, 3), (p3, 0, 4))):
                        for i in range(4):
                            s = h * 4 + i
                            nc.tensor.matmul(wt[:, i, :], pw[:],
                                             conv_rhs(ms, s, c0, st),
                                             start=(wi == 0),
                                             stop=(wi == 2),
                                             perf_mode=DR)
                    nc.vector._custom_dve(
                        BOUND, out=junk[:],
                        in0=wt[:], in1=nb[:, h * 4:(h + 1) * 4, :],
                        s0=24.0, accum_out=accb[:, 2 * g + h:2 * g + h + 1])
                state[g] = (ts2, pt)

            def consume(g):
                ts2, pt = state.pop(g)
                for s in range(GS):
                    for k in range(4):
                        nc.tensor.matmul(
                            dice_ps[:],
                            ts2[:, s, k * P:(k + 1) * P],
                            pt[:, s, k * P:(k + 1) * P],
                            start=(g == 0 and s == 0 and k == 0),
                            stop=(g == GROUPS - 1 and s == GS - 1
                                  and k == 3))

            def aux():
                for h in range(2):
                    mctx, xw, tw = auxt[h]
                    zw = ax.tile([48, W], bf16, tag="zw")
                    nc.vector.scalar_tensor_tensor(
                        out=zw[:], in0=tw[:], scalar=0.5, in1=xw[:],
                        op0=Alu.subtract, op1=Alu.mult)
                    qw = ax.tile([48, W], bf16, tag="qw")
                    bw = ax.tile([48, W], bf16, tag="bw")
                    nc.scalar.activation(qw[:], zw[:], Act.Exp, scale=-2.0)
                    nc.scalar.activation(bw[:], qw[:], Act.Ln, bias=1.0)
                    jw = ax.tile([48, W], bf16, tag="jw")
                    base = mctx[:, :]
                    pd = list(base.ap[0])
                    for op, col, mats in (
                        (BOUND, h, ("a1", "a2", "a3")),
                        (NBOUND, 2 + h, ("a1t", "a2t", "a3t")),
                    ):
                        wt = psa.tile([48, W], f32, tag="wtrue")
                        for wi, (wk, c0, st) in enumerate(
                                ((mats[0], 1, 1), (mats[1], 0, 3),
                                 (mats[2], 0, 4))):
                            rhs = AP(base.tensor, base.offset + c0,
                                     [pd, [st, 2], [1, W]])
                            nc.tensor.matmul(wt[:], auxw[wk][:], rhs,
                                             start=(wi == 0),
                                             stop=(wi == 2),
                                             perf_mode=DR)
                        nc.vector._custom_dve(
                            op, out=jw[:], in0=wt[:], in1=bw[:], s0=24.0,
                            accum_out=acca[:, col:col + 1])

            produce(0)
            produce(1)
            consume(0)
            produce(2)
            consume(1)
            produce(3)
            aux()
            consume(2)
            consume(3)

            nc.sync.dma_start(acc_d[:], acc[:])
            nc.sync.dma_start(accb_d[:], accb[:])
            nc.sync.dma_start(acca_d[:], acca[:])
            dsb = cn.tile([P, P], f32, tag="dsb")
            nc.scalar.copy(dsb[:], dice_ps[:])
            nc.sync.dma_start(dice_d[:], dsb[:])

    nc.compile()
    unpatch()
    return nc


def _get_nc():
    if "nc" not in _CACHE:
        _CACHE["nc"] = _build()
    return _CACHE["nc"]


def kernel(inputs: np.ndarray, targets: np.ndarray) -> np.ndarray:
    import os
    import ml_dtypes
    from concourse.bass_utils import run_bass_kernel_spmd

    bf = ml_dtypes.bfloat16
    f8 = ml_dtypes.float8_e4m3

    nc = _get_nc()
    st = _stationaries()

    x = np.asarray(inputs, dtype=np.float32).reshape(64, H, W)
    t = np.asarray(targets, dtype=np.float32).reshape(64, H, W)

    in_maps = []
    for c in range(N_CORES):
        xc = x[c * IMG:(c + 1) * IMG]     # [8, 512, 512]
        tc_ = t[c * IMG:(c + 1) * IMG]
        # transposed layout: [128 p=row-in-band, slot=img*4+band, 512]
        xT = np.ascontiguousarray(
            xc.reshape(IMG, BANDS, P, W).transpose(2, 0, 1, 3)
            .reshape(P, SLOTS, W)).astype(bf)
        tT = (tc_.reshape(IMG, BANDS, P, W).transpose(2, 0, 1, 3)
              .reshape(P, SLOTS, W))
        t2 = np.ascontiguousarray(32.0 * (tT - 0.5)).astype(bf)
        mp = np.zeros((P, SLOTS, W + 4), np.float32)
        mp[:, :, 2:2 + W] = tT
        im = {"x": xT, "t2": t2, "m": mp.astype(f8)}
        # aux: ctx rows 124+128b..131+128b, wrong rows 126+128b..129+128b
        for h in range(2):
            imgs = tc_[4 * h:4 * h + 4]
            ximgs = xc[4 * h:4 * h + 4]
            tctx = np.stack([imgs[li, 124 + 128 * b:132 + 128 * b, :]
                             for li in range(4) for b in range(3)])
            mctx = np.zeros((96, W + 4), np.float32)
            mctx[:, 2:2 + W] = tctx.reshape(96, W)
            im[f"mctx{h}"] = mctx.astype(f8)
            twr = np.stack([imgs[li, 126 + 128 * b:130 + 128 * b, :]
                            for li in range(4) for b in range(3)])
            im[f"twr{h}"] = np.ascontiguousarray(
                twr.reshape(48, W)).astype(bf)
            xwr = np.stack([ximgs[li, 126 + 128 * b:130 + 128 * b, :]
                            for li in range(4) for b in range(3)])
            im[f"xwr{h}"] = np.ascontiguousarray(
                xwr.reshape(48, W)).astype(bf)
        im.update(st)
        in_maps.append(im)

    trace = bool(os.environ.get("BASS_TRACE_KERNEL"))
    res = run_bass_kernel_spmd(nc, in_maps, core_ids=list(range(N_CORES)),
                               trace=trace)
    _CACHE["exec_time_ns"] = res.exec_time_ns

    s_bce = s_pt = s_focal = s_bnd = s_diag = 0.0
    for c in range(N_CORES):
        acc = res.results[c]["acc"].astype(np.float64)
        s_bce += acc[:, 0:4].sum()
        s_pt += acc[:, 4:8].sum()
        s_focal += acc[:, 8:12].sum()
        s_bnd += res.results[c]["accb"].astype(np.float64).sum()
        s_bnd += res.results[c]["acca"].astype(np.float64).sum()
        s_diag += np.trace(res.results[c]["dice"].astype(np.float64))

    n = float(64 * H * W)
    s_tpt = (s_diag + 16.0 * s_pt) / 32.0
    focal_loss = 0.25 * s_focal / n
    denom = n - s_pt + 2.0 * s_tpt
    dice = (2.0 * s_tpt + 1e-6) / (denom + 1e-6)
    dice_loss = 1.0 - dice
    boundary_loss = (s_bce + 5.0 * s_bnd) / n
    loss = 0.3 * focal_loss + 0.4 * dice_loss + 0.3 * boundary_loss
    return np.float32(loss)


# revision 20
# speedup vs baseline: 1.6626x; 1.0200x over previous
"""Trainium2 Bass kernel for CombinedLoss (focal + dice + boundary-weighted BCE).

Contract: kernel(inputs, targets) takes FULL (64,1,512,512) fp32 arrays and
returns the full scalar loss (fp32). Data-parallel over batch: 8 images per
NeuronCore on 8 cores; host combines per-core partial sums in float64.

Design (engine-balanced, transposed layout [128 p = row-in-band,
32 slots = img*4+band, 512 cols], bf16/fp8 inputs):
  Host sends x (bf16), t2 = 32*(t-0.5) (bf16, exact), and m (t zero-padded
  2 cols each side, fp8e4).
  zh = t2*x (DVE TT, 2x bf16 mode); q = exp(-zh/16); bce = ln(1+q) +acc;
  pt = exp(-bce) +acc  (ScalarE, single pinned exp/ln table set).
  FOCAL custom DVE op: (1-pt)^2*bce, accum.
  Morphology: 2-iter erode/dilate == thresholds of W = conv2(m, 13-pt
  diamond). 5 taps computed as 3 fp8 DoubleRow matmuls per slot (two
  vertical-band stationaries per instruction, shifted moving views of the
  padded m). BOUND custom DVE op on PSUM W: (min(W,1)-relu(W-24))*bce, acc.
  Rows 0,1,126,127 at the 3 internal band boundaries of each image get
  wrong W from band truncation; the aux pass (block-diag stationaries
  S5/S3/S1 true + truncated) cancels it exactly via BOUND/NBOUND on 96 rows.
  dice sum(t*pt): GpSimd (otherwise idle) scalar_tensor_tensor
  (t2/32)*pt with accum = Sum((t-0.5)*pt); zh also on GpSimd. sum(t) is
  never needed: Sum(p)+Sum(t) = n - Sum(pt) + 2*Sum(t*pt) (cancellation),
  and Sum(t*pt) = Sum((t-0.5)*pt) + 0.5*Sum(pt).
"""

import numpy as np
import operator

N_CORES = 8
IMG = 8            # images per core
H = 512
W = 512
BANDS = 4          # 128-row bands per image
P = 128
SLOTS = IMG * BANDS   # 32, slot = img*4 + band
GROUPS = 4
GS = SLOTS // GROUPS  # 8 slots per group = 2 images

_CACHE = {}


def _register_dve_op(name, spec):
    from concourse import dve_ops
    from concourse.dve_uop import DveOpSpec
    from concourse.dve_spec import lower
    for op in dve_ops.OPS:
        if op.name == name:
            return op
    opcode = max(dve_ops._SUB_OPCODE_FOR_NAME.values()) + 1
    assert opcode < 0x20
    dve_ops._SUB_OPCODE_FOR_NAME[name] = opcode
    uops = lower(spec, ver="v3")
    sha = DveOpSpec(name=name, opcode=opcode, uops=uops,
                    rd1_en=dve_ops.has_src1(spec)).sha("v3")
    op = dve_ops.DveOp(name, spec, subdim=False, uops_sha={"v3": sha})
    dve_ops.OPS.append(op)
    return op


def _stationaries():
    """Conv stationaries: fp8 DoubleRow pairs + aux block-diag bf16."""
    import ml_dtypes
    bf = ml_dtypes.bfloat16
    f8 = ml_dtypes.float8_e4m3
    kv0 = [1.0, 2.0, 5.0, 2.0, 1.0]   # dc=0 column of the diamond kernel
    kv1 = [2.0, 2.0, 2.0]             # dc=+-1
    B5 = np.zeros((P, P), np.float32)
    B3 = np.zeros((P, P), np.float32)
    for p in range(P):
        for i in range(max(0, p - 2), min(P, p + 3)):
            B5[p, i] = kv0[p - i + 2]
        for i in range(max(0, p - 1), min(P, p + 2)):
            B3[p, i] = kv1[p - i + 1]
    B1 = np.eye(P, dtype=np.float32)
    Z = np.zeros((P, P), np.float32)
    # DoubleRow pairs [K, 2, M]; member i pairs with moving view i.
    # p1: cols (+1, +2) -> (B3*m_-1, B5*m_0); p2: cols (+0, +3) ->
    # (B1*m_-2, B3*m_+1); p3: cols (+0, +4) -> (0, B1*m_+2).
    p1 = np.stack([B3, B5], axis=1)
    p2 = np.stack([B1, B3], axis=1)
    p3 = np.stack([Z, B1], axis=1)
    # aux block-diag: q=(li,b,k ctx row 0..7) -> j=(li,b,w wrong row 0..3)
    # ctx row k = img row 124+128b+k ; wrong row w = img row 126+128b+w
    # vertical delta = k - w - 2
    S5 = np.zeros((96, 48), np.float32)
    S3 = np.zeros((96, 48), np.float32)
    S1 = np.zeros((96, 48), np.float32)
    S5t = np.zeros((96, 48), np.float32)
    S3t = np.zeros((96, 48), np.float32)
    S1t = np.zeros((96, 48), np.float32)
    for li in range(4):
        for b in range(3):
            for k in range(8):
                for w in range(4):
                    d = k - w - 2
                    q = li * 24 + b * 8 + k
                    j = li * 12 + b * 4 + w
                    same = (w < 2 and k < 4) or (w >= 2 and k >= 4)
                    if -2 <= d <= 2:
                        S5[q, j] = kv0[d + 2]
                        if same:
                            S5t[q, j] = kv0[d + 2]
                    if -1 <= d <= 1:
                        S3[q, j] = kv1[d + 1]
                        if same:
                            S3t[q, j] = kv1[d + 1]
                    if d == 0:
                        S1[q, j] = 1.0
                        if same:
                            S1t[q, j] = 1.0
    Z48 = np.zeros((96, 48), np.float32)
    out = {k: v.astype(f8) for k, v in dict(
        p1=p1, p2=p2, p3=p3,
        a1=np.stack([S3, S5], axis=1),
        a2=np.stack([S1, S3], axis=1),
        a3=np.stack([Z48, S1], axis=1),
        a1t=np.stack([S3t, S5t], axis=1),
        a2t=np.stack([S1t, S3t], axis=1),
        a3t=np.stack([Z48, S1t], axis=1)).items()}
    del bf
    return out


def _patch_act_tables():
    """Pin exp/ln/copy activations to the one table set containing all of
    them (natural_log_exp_and_others) so the kernel does a single
    ACT_TABLE_LOAD instead of thrashing between per-function sets."""
    from concourse import bacc as bacc_mod, hw_specs
    orig = hw_specs.get_activation_tables
    keep = "natural_log_exp_and_others"

    def patched(arch):
        t = orig(arch)
        pin = set(t[keep])
        return {k: (v if k == keep else {f for f in v if f not in pin})
                for k, v in t.items()}

    bacc_mod.get_activation_tables = patched
    return lambda: setattr(bacc_mod, "get_activation_tables", orig)


def _build():
    from concourse import bacc, mybir, tile
    from bass_rust import AP
    from concourse.dve_spec import (Spec, Src0, Src1, C0, One, Zero,
                                    minn, maxx, sq)

    f32 = mybir.dt.float32
    bf16 = mybir.dt.bfloat16
    fp8 = mybir.dt.float8e4
    Alu = mybir.AluOpType
    Act = mybir.ActivationFunctionType
    DR = mybir.MatmulPerfMode.DoubleRow

    FOCAL = _register_dve_op("ANT_FOCAL_SSQ", Spec(
        body=sq(One - Src0) * Src1, accum=operator.add))
    BOUND = _register_dve_op("ANT_BOUND_WDF", Spec(
        body=(minn(Src0, One) - maxx(Src0 - C0, Zero)) * Src1,
        accum=operator.add))
    NBOUND = _register_dve_op("ANT_BOUND_NEG", Spec(
        body=(maxx(Src0 - C0, Zero) - minn(Src0, One)) * Src1,
        accum=operator.add))

    unpatch = _patch_act_tables()
    nc = bacc.Bacc("TRN2", target_bir_lowering=False, debug=False,
                   num_devices=N_CORES)

    x_d = nc.dram_tensor("x", [P, SLOTS, W], bf16, kind="ExternalInput").ap()
    t2_d = nc.dram_tensor("t2", [P, SLOTS, W], bf16,
                          kind="ExternalInput").ap()
    m_d = nc.dram_tensor("m", [P, SLOTS, W + 4], fp8,
                         kind="ExternalInput").ap()
    mctx_d = [nc.dram_tensor(f"mctx{h}", [96, W + 4], fp8,
                             kind="ExternalInput").ap() for h in range(2)]
    xwr_d = [nc.dram_tensor(f"xwr{h}", [48, W], bf16,
                            kind="ExternalInput").ap() for h in range(2)]
    twr_d = [nc.dram_tensor(f"twr{h}", [48, W], bf16,
                            kind="ExternalInput").ap() for h in range(2)]
    p1_d = nc.dram_tensor("p1", [P, 2, P], fp8, kind="ExternalInput").ap()
    p2_d = nc.dram_tensor("p2", [P, 2, P], fp8, kind="ExternalInput").ap()
    p3_d = nc.dram_tensor("p3", [P, 2, P], fp8, kind="ExternalInput").ap()
    aux_d = {k: nc.dram_tensor(k, [96, 2, 48], fp8, kind="ExternalInput").ap()
             for k in ("a1", "a2", "a3", "a1t", "a2t", "a3t")}

    # acc cols: 0:4 sum(bce) per group, 4:8 sum(pt), 8:12 focal sum,
    # 12:16 sum((t-0.5)*pt)
    acc_d = nc.dram_tensor("acc", [P, 16], f32, kind="ExternalOutput").ap()
    accb_d = nc.dram_tensor("accb", [P, 8], f32, kind="ExternalOutput").ap()
    acca_d = nc.dram_tensor("acca", [48, 4], f32, kind="ExternalOutput").ap()
    dice_d = nc.dram_tensor("dice", [P, P], f32, kind="ExternalOutput").ap()

    with tile.TileContext(nc) as tc:
        with (
            tc.tile_pool(name="io", bufs=3) as io,
            tc.tile_pool(name="cn", bufs=1) as cn,
            tc.tile_pool(name="ew", bufs=2) as ew,
            tc.tile_pool(name="jk", bufs=1) as jk,
            tc.tile_pool(name="ax", bufs=1) as ax,
            tc.tile_pool(name="psw", bufs=1, space="PSUM") as psw,
            tc.tile_pool(name="psd", bufs=1, space="PSUM") as psd,
            tc.tile_pool(name="psa", bufs=1, space="PSUM") as psa,
        ):
            p1 = cn.tile([P, 2, P], fp8, tag="p1")
            p2 = cn.tile([P, 2, P], fp8, tag="p2")
            p3 = cn.tile([P, 2, P], fp8, tag="p3")
            auxw = {}
            for k in ("a1", "a2", "a3", "a1t", "a2t", "a3t"):
                auxw[k] = cn.tile([96, 2, 48], fp8, tag=k, name=k)
                nc.sync.dma_start(auxw[k][:], aux_d[k][:])
            for tl, dd in ((p1, p1_d), (p2, p2_d), (p3, p3_d)):
                nc.sync.dma_start(tl[:], dd[:])

            # acc cols: chunk c (5 chunks 0a,0b,1,2,3): bce c, pt 5+c,
            # focal 10+c
            acc = cn.tile([P, 16], f32, tag="acc")
            accb = cn.tile([P, 8], f32, tag="accb")
            acca = cn.tile([48, 4], f32, tag="acca")
            nc.vector.memset(acc[:], 0.0)
            nc.vector.memset(accb[:], 0.0)
            nc.vector.memset(acca[:], 0.0)

            dice_ps = psd.tile([P, P], f32, tag="dice")

            # aux inputs loaded early so the tail compute never waits on DMA
            auxt = []
            for h in range(2):
                mctx = ax.tile([96, W + 4], fp8, tag=f"mctx{h}")
                xw = ax.tile([48, W], bf16, tag=f"xw{h}")
                tw = ax.tile([48, W], bf16, tag=f"tw{h}")
                nc.sync.dma_start(mctx[:], mctx_d[h][:])
                nc.sync.dma_start(xw[:], xwr_d[h][:])
                nc.sync.dma_start(tw[:], twr_d[h][:])
                auxt.append((mctx, xw, tw))

            # chunks: (slot0, nslots); 0a/0b fine-grained to cut startup
            CH = [(0, 4), (4, 4), (8, 8), (16, 8), (24, 8)]
            NCH = len(CH)
            tiles = {}

            def conv_rhs(ms, s, c0, step):
                base = ms[:, s, :]
                pd = list(base.ap[0])
                return AP(base.tensor, base.offset + c0,
                          [pd, [step, 2], [1, W]])

            def dma(c):
                s0, ns = CH[c]
                xs = io.tile([P, ns, W], bf16, tag=f"xs{ns}", name=f"xs{c}")
                ts2 = io.tile([P, ns, W], bf16, tag=f"ts{ns}", name=f"ts{c}")
                ms = io.tile([P, ns, W + 4], fp8, tag=f"ms{ns}",
                             name=f"ms{c}")
                h = ns // 2
                nc.sync.dma_start(xs[:, 0:h, :], x_d[:, s0:s0 + h, :])
                nc.sync.dma_start(xs[:, h:ns, :], x_d[:, s0 + h:s0 + ns, :])
                nc.sync.dma_start(ts2[:, 0:h, :], t2_d[:, s0:s0 + h, :])
                nc.sync.dma_start(ts2[:, h:ns, :],
                                  t2_d[:, s0 + h:s0 + ns, :])
                nc.sync.dma_start(ms[:], m_d[:, s0:s0 + ns, :])
                tiles[c] = [xs, ts2, ms]

            def zh_op(c):
                s0, ns = CH[c]
                xs, ts2, ms = tiles[c]
                zh = ew.tile([P, ns, W], bf16, tag=f"zh{ns}", name=f"zh{c}")
                nc.vector.tensor_tensor(out=zh[:], in0=ts2[:], in1=xs[:],
                                        op=Alu.mult)
                tiles[c].append(zh)

            def acts(c):
                s0, ns = CH[c]
                xs, ts2, ms, zh = tiles[c]
                # q overwrites xs (dead after zh) to save SBUF
                q = xs
                nc.scalar.activation(q[:], zh[:], Act.Exp, scale=-0.0625)
                nb = ew.tile([P, ns, W], bf16, tag=f"nb{ns}", name=f"nb{c}")
                nc.scalar.activation(nb[:], q[:], Act.Ln, bias=1.0,
                                     accum_out=acc[:, c:c + 1])
                pt = ew.tile([P, ns, W], bf16, tag=f"pt{ns}", name=f"pt{c}")
                nc.scalar.activation(pt[:], nb[:], Act.Exp, scale=-1.0,
                                     accum_out=acc[:, 5 + c:6 + c])
                tiles[c] += [nb, pt]

            def conv(c, sb_base):
                s0, ns = CH[c]
                ms = tiles[c][2]
                for b in range(ns // 4):
                    wt = psw.tile([P, 4, W], f32, tag="wt",
                                  name=f"W_c{c}_b{b}")
                    for wi, (pw, c0, st) in enumerate(
                            ((p1, 1, 1), (p2, 0, 3), (p3, 0, 4))):
                        for i in range(4):
                            s = b * 4 + i
                            nc.tensor.matmul(wt[:, i, :], pw[:],
                                             conv_rhs(ms, s, c0, st),
                                             start=(wi == 0),
                                             stop=(wi == 2),
                                             perf_mode=DR)
                    tiles[c].append((wt, b, sb_base + b))

            def focal(c):
                s0, ns = CH[c]
                nb, pt = tiles[c][4], tiles[c][5]
                junk2 = jk.tile([P, ns, W], bf16, tag=f"junk2{ns}")
                nc.vector._custom_dve(
                    FOCAL, out=junk2[:], in0=pt[:], in1=nb[:],
                    accum_out=acc[:, 10 + c:11 + c])

            def bound(c):
                s0, ns = CH[c]
                nb = tiles[c][4]
                junk = jk.tile([P, 4, W], bf16, tag="junk")
                for wt, b, col in tiles[c][6:]:
                    nc.vector._custom_dve(
                        BOUND, out=junk[:],
                        in0=wt[:], in1=nb[:, b * 4:(b + 1) * 4, :],
                        s0=24.0, accum_out=accb[:, col:col + 1])

            def diag(c):
                s0, ns = CH[c]
                ts2, pt = tiles[c][1], tiles[c][5]
                for s in range(ns):
                    for k in range(4):
                        nc.tensor.matmul(
                            dice_ps[:],
                            ts2[:, s, k * P:(k + 1) * P],
                            pt[:, s, k * P:(k + 1) * P],
                            start=(c == 0 and s == 0 and k == 0),
                            stop=(c == NCH - 1 and s == ns - 1 and k == 3))

            def aux():
                for h in range(2):
                    mctx, xw, tw = auxt[h]
                    zw = ax.tile([48, W], bf16, tag="zw")
                    nc.vector.scalar_tensor_tensor(
                        out=zw[:], in0=tw[:], scalar=0.5, in1=xw[:],
                        op0=Alu.subtract, op1=Alu.mult)
                    qw = ax.tile([48, W], bf16, tag="qw")
                    bw = ax.tile([48, W], bf16, tag="bw")
                    nc.scalar.activation(qw[:], zw[:], Act.Exp, scale=-2.0)
                    nc.scalar.activation(bw[:], qw[:], Act.Ln, bias=1.0)
                    jw = ax.tile([48, W], bf16, tag="jw")
                    base = mctx[:, :]
                    pd = list(base.ap[0])
                    for op, col, mats in (
                        (BOUND, h, ("a1", "a2", "a3")),
                        (NBOUND, 2 + h, ("a1t", "a2t", "a3t")),
                    ):
                        wt = psa.tile([48, W], f32, tag="wtrue")
                        for wi, (wk, c0, st) in enumerate(
                                ((mats[0], 1, 1), (mats[1], 0, 3),
                                 (mats[2], 0, 4))):
                            rhs = AP(base.tensor, base.offset + c0,
                                     [pd, [st, 2], [1, W]])
                            nc.tensor.matmul(wt[:], auxw[wk][:], rhs,
                                             start=(wi == 0),
                                             stop=(wi == 2),
                                             perf_mode=DR)
                        nc.vector._custom_dve(
                            op, out=jw[:], in0=wt[:], in1=bw[:], s0=24.0,
                            accum_out=acca[:, col:col + 1])

            for c in range(NCH):
                dma(c)
            zh_op(0)
            zh_op(1)
            acts(0)
            conv(0, 0)
            zh_op(2)
            acts(1)
            conv(1, 1)
            focal(0)
            bound(0)
            acts(2)
            conv(2, 2)
            zh_op(3)
            focal(1)
            bound(1)
            diag(0)
            acts(3)
            conv(3, 4)
            focal(2)
            bound(2)
            diag(1)
            aux()
            zh_op(4)
            diag(2)
            acts(4)
            conv(4, 6)
            focal(3)
            bound(3)
            focal(4)
            bound(4)
            diag(3)
            diag(4)

            nc.sync.dma_start(acc_d[:], acc[:])
            nc.sync.dma_start(accb_d[:], accb[:])
            nc.sync.dma_start(acca_d[:], acca[:])
            dsb = cn.tile([P, P], f32, tag="dsb")
            nc.scalar.copy(dsb[:], dice_ps[:])
            nc.sync.dma_start(dice_d[:], dsb[:])

    nc.compile()
    unpatch()
    return nc


def _get_nc():
    if "nc" not in _CACHE:
        _CACHE["nc"] = _build()
    return _CACHE["nc"]


def kernel(inputs: np.ndarray, targets: np.ndarray) -> np.ndarray:
    import os
    import ml_dtypes
    from concourse.bass_utils import run_bass_kernel_spmd

    bf = ml_dtypes.bfloat16
    f8 = ml_dtypes.float8_e4m3

    nc = _get_nc()
    st = _stationaries()

    x = np.asarray(inputs, dtype=np.float32).reshape(64, H, W)
    t = np.asarray(targets, dtype=np.float32).reshape(64, H, W)

    in_maps = []
    for c in range(N_CORES):
        xc = x[c * IMG:(c + 1) * IMG]     # [8, 512, 512]
        tc_ = t[c * IMG:(c + 1) * IMG]
        # transposed layout: [128 p=row-in-band, slot=img*4+band, 512]
        xT = np.ascontiguousarray(
            xc.reshape(IMG, BANDS, P, W).transpose(2, 0, 1, 3)
            .reshape(P, SLOTS, W)).astype(bf)
        tT = (tc_.reshape(IMG, BANDS, P, W).transpose(2, 0, 1, 3)
              .reshape(P, SLOTS, W))
        t2 = np.ascontiguousarray(32.0 * (tT - 0.5)).astype(bf)
        mp = np.zeros((P, SLOTS, W + 4), np.float32)
        mp[:, :, 2:2 + W] = tT
        im = {"x": xT, "t2": t2, "m": mp.astype(f8)}
        # aux: ctx rows 124+128b..131+128b, wrong rows 126+128b..129+128b
        for h in range(2):
            imgs = tc_[4 * h:4 * h + 4]
            ximgs = xc[4 * h:4 * h + 4]
            tctx = np.stack([imgs[li, 124 + 128 * b:132 + 128 * b, :]
                             for li in range(4) for b in range(3)])
            mctx = np.zeros((96, W + 4), np.float32)
            mctx[:, 2:2 + W] = tctx.reshape(96, W)
            im[f"mctx{h}"] = mctx.astype(f8)
            twr = np.stack([imgs[li, 126 + 128 * b:130 + 128 * b, :]
                            for li in range(4) for b in range(3)])
            im[f"twr{h}"] = np.ascontiguousarray(
                twr.reshape(48, W)).astype(bf)
            xwr = np.stack([ximgs[li, 126 + 128 * b:130 + 128 * b, :]
                            for li in range(4) for b in range(3)])
            im[f"xwr{h}"] = np.ascontiguousarray(
                xwr.reshape(48, W)).astype(bf)
        im.update(st)
        in_maps.append(im)

    trace = bool(os.environ.get("BASS_TRACE_KERNEL"))
    res = run_bass_kernel_spmd(nc, in_maps, core_ids=list(range(N_CORES)),
                               trace=trace)
    _CACHE["exec_time_ns"] = res.exec_time_ns

    s_bce = s_pt = s_focal = s_bnd = s_diag = 0.0
    for c in range(N_CORES):
        acc = res.results[c]["acc"].astype(np.float64)
        s_bce += acc[:, 0:5].sum()
        s_pt += acc[:, 5:10].sum()
        s_focal += acc[:, 10:15].sum()
        s_bnd += res.results[c]["accb"].astype(np.float64).sum()
        s_bnd += res.results[c]["acca"].astype(np.float64).sum()
        s_diag += np.trace(res.results[c]["dice"].astype(np.float64))

    n = float(64 * H * W)
    s_tpt = (s_diag + 16.0 * s_pt) / 32.0
    focal_loss = 0.25 * s_focal / n
    denom = n - s_pt + 2.0 * s_tpt
    dice = (2.0 * s_tpt + 1e-6) / (denom + 1e-6)
    dice_loss = 1.0 - dice
    boundary_loss = (s_bce + 5.0 * s_bnd) / n
    loss = 0.3 * focal_loss + 0.4 * dice_loss + 0.3 * boundary_loss
    return np.float32(loss)


# revision 22
# speedup vs baseline: 1.7224x; 1.0360x over previous
"""Trainium2 Bass kernel for CombinedLoss (focal + dice + boundary-weighted BCE).

Contract: kernel(inputs, targets) takes FULL (64,1,512,512) fp32 arrays and
returns the full scalar loss (fp32). Data-parallel over batch: 8 images per
NeuronCore on 8 cores; host combines per-core partial sums in float64.

Design (engine-balanced, transposed layout [128 p = row-in-band,
32 slots = img*4+band, 512 cols], bf16/fp8 inputs):
  Host sends x (bf16), t2 = 32*(t-0.5) (bf16, exact), and m (t zero-padded
  2 cols each side, fp8e4).
  zh = t2*x (DVE TT, 2x bf16 mode); q = exp(-zh/16); bce = ln(1+q) +acc;
  pt = exp(-bce) +acc  (ScalarE, single pinned exp/ln table set).
  FOCAL custom DVE op: (1-pt)^2*bce, accum.
  Morphology: 2-iter erode/dilate == thresholds of W = conv2(m, 13-pt
  diamond). 5 taps computed as 3 fp8 DoubleRow matmuls per slot (two
  vertical-band stationaries per instruction, shifted moving views of the
  padded m). BOUND custom DVE op on PSUM W: (min(W,1)-relu(W-24))*bce, acc.
  Rows 0,1,126,127 at the 3 internal band boundaries of each image get
  wrong W from band truncation; the aux pass (block-diag stationaries
  S5/S3/S1 true + truncated) cancels it exactly via BOUND/NBOUND on 96 rows.
  dice sum(t*pt): GpSimd (otherwise idle) scalar_tensor_tensor
  (t2/32)*pt with accum = Sum((t-0.5)*pt); zh also on GpSimd. sum(t) is
  never needed: Sum(p)+Sum(t) = n - Sum(pt) + 2*Sum(t*pt) (cancellation),
  and Sum(t*pt) = Sum((t-0.5)*pt) + 0.5*Sum(pt).
"""

import numpy as np
import operator

N_CORES = 8
IMG = 8            # images per core
H = 512
W = 512
BANDS = 4          # 128-row bands per image
P = 128
SLOTS = IMG * BANDS   # 32, slot = img*4 + band
GROUPS = 4
GS = SLOTS // GROUPS  # 8 slots per group = 2 images

_CACHE = {}


def _register_dve_op(name, spec):
    from concourse import dve_ops
    from concourse.dve_uop import DveOpSpec
    from concourse.dve_spec import lower
    for op in dve_ops.OPS:
        if op.name == name:
            return op
    opcode = max(dve_ops._SUB_OPCODE_FOR_NAME.values()) + 1
    assert opcode < 0x20
    dve_ops._SUB_OPCODE_FOR_NAME[name] = opcode
    uops = lower(spec, ver="v3")
    sha = DveOpSpec(name=name, opcode=opcode, uops=uops,
                    rd1_en=dve_ops.has_src1(spec)).sha("v3")
    op = dve_ops.DveOp(name, spec, subdim=False, uops_sha={"v3": sha})
    dve_ops.OPS.append(op)
    return op


def _stationaries():
    """Conv stationaries: fp8 DoubleRow pairs + aux block-diag bf16."""
    import ml_dtypes
    bf = ml_dtypes.bfloat16
    f8 = ml_dtypes.float8_e4m3
    kv0 = [1.0, 2.0, 5.0, 2.0, 1.0]   # dc=0 column of the diamond kernel
    kv1 = [2.0, 2.0, 2.0]             # dc=+-1
    B5 = np.zeros((P, P), np.float32)
    B3 = np.zeros((P, P), np.float32)
    for p in range(P):
        for i in range(max(0, p - 2), min(P, p + 3)):
            B5[p, i] = kv0[p - i + 2]
        for i in range(max(0, p - 1), min(P, p + 2)):
            B3[p, i] = kv1[p - i + 1]
    B1 = np.eye(P, dtype=np.float32)
    Z = np.zeros((P, P), np.float32)
    # DoubleRow pairs [K, 2, M]; member i pairs with moving view i.
    # p1: cols (+1, +2) -> (B3*m_-1, B5*m_0); p2: cols (+0, +3) ->
    # (B1*m_-2, B3*m_+1); p3: cols (+0, +4) -> (0, B1*m_+2).
    p1 = np.stack([B3, B5], axis=1)
    p2 = np.stack([B1, B3], axis=1)
    p3 = np.stack([Z, B1], axis=1)
    # aux block-diag: q=(li,b,k ctx row 0..7) -> j=(li,b,w wrong row 0..3)
    # ctx row k = img row 124+128b+k ; wrong row w = img row 126+128b+w
    # vertical delta = k - w - 2
    S5 = np.zeros((96, 48), np.float32)
    S3 = np.zeros((96, 48), np.float32)
    S1 = np.zeros((96, 48), np.float32)
    S5t = np.zeros((96, 48), np.float32)
    S3t = np.zeros((96, 48), np.float32)
    S1t = np.zeros((96, 48), np.float32)
    for li in range(4):
        for b in range(3):
            for k in range(8):
                for w in range(4):
                    d = k - w - 2
                    q = li * 24 + b * 8 + k
                    j = li * 12 + b * 4 + w
                    same = (w < 2 and k < 4) or (w >= 2 and k >= 4)
                    if -2 <= d <= 2:
                        S5[q, j] = kv0[d + 2]
                        if same:
                            S5t[q, j] = kv0[d + 2]
                    if -1 <= d <= 1:
                        S3[q, j] = kv1[d + 1]
                        if same:
                            S3t[q, j] = kv1[d + 1]
                    if d == 0:
                        S1[q, j] = 1.0
                        if same:
                            S1t[q, j] = 1.0
    Z48 = np.zeros((96, 48), np.float32)
    out = {k: v.astype(f8) for k, v in dict(
        p1=p1, p2=p2, p3=p3,
        a1=np.stack([S3, S5], axis=1),
        a2=np.stack([S1, S3], axis=1),
        a3=np.stack([Z48, S1], axis=1),
        a1t=np.stack([S3t, S5t], axis=1),
        a2t=np.stack([S1t, S3t], axis=1),
        a3t=np.stack([Z48, S1t], axis=1)).items()}
    del bf
    return out


def _patch_act_tables():
    """Pin exp/ln/copy activations to the one table set containing all of
    them (natural_log_exp_and_others) so the kernel does a single
    ACT_TABLE_LOAD instead of thrashing between per-function sets."""
    from concourse import bacc as bacc_mod, hw_specs
    orig = hw_specs.get_activation_tables
    keep = "natural_log_exp_and_others"

    def patched(arch):
        t = orig(arch)
        pin = set(t[keep])
        return {k: (v if k == keep else {f for f in v if f not in pin})
                for k, v in t.items()}

    bacc_mod.get_activation_tables = patched
    return lambda: setattr(bacc_mod, "get_activation_tables", orig)


def _build():
    from concourse import bacc, mybir, tile
    from bass_rust import AP
    from concourse.dve_spec import (Spec, Src0, Src1, C0, One, Zero,
                                    minn, maxx, sq)

    f32 = mybir.dt.float32
    bf16 = mybir.dt.bfloat16
    fp8 = mybir.dt.float8e4
    Alu = mybir.AluOpType
    Act = mybir.ActivationFunctionType
    DR = mybir.MatmulPerfMode.DoubleRow

    FOCAL = _register_dve_op("ANT_FOCAL_SSQ", Spec(
        body=sq(One - Src0) * Src1, accum=operator.add))
    BOUND = _register_dve_op("ANT_BOUND_WDF", Spec(
        body=(minn(Src0, One) - maxx(Src0 - C0, Zero)) * Src1,
        accum=operator.add))
    NBOUND = _register_dve_op("ANT_BOUND_NEG", Spec(
        body=(maxx(Src0 - C0, Zero) - minn(Src0, One)) * Src1,
        accum=operator.add))

    unpatch = _patch_act_tables()
    nc = bacc.Bacc("TRN2", target_bir_lowering=False, debug=False,
                   num_devices=N_CORES)

    x_d = nc.dram_tensor("x", [P, SLOTS, W], bf16, kind="ExternalInput").ap()
    t2_d = nc.dram_tensor("t2", [P, SLOTS, W], bf16,
                          kind="ExternalInput").ap()
    m_d = nc.dram_tensor("m", [P, SLOTS, W + 4], fp8,
                         kind="ExternalInput").ap()
    mctx_d = [nc.dram_tensor(f"mctx{h}", [96, W + 4], fp8,
                             kind="ExternalInput").ap() for h in range(2)]
    xwr_d = [nc.dram_tensor(f"xwr{h}", [48, W], bf16,
                            kind="ExternalInput").ap() for h in range(2)]
    twr_d = [nc.dram_tensor(f"twr{h}", [48, W], bf16,
                            kind="ExternalInput").ap() for h in range(2)]
    p1_d = nc.dram_tensor("p1", [P, 2, P], fp8, kind="ExternalInput").ap()
    p2_d = nc.dram_tensor("p2", [P, 2, P], fp8, kind="ExternalInput").ap()
    p3_d = nc.dram_tensor("p3", [P, 2, P], fp8, kind="ExternalInput").ap()
    aux_d = {k: nc.dram_tensor(k, [96, 2, 48], fp8, kind="ExternalInput").ap()
             for k in ("a1", "a2", "a3", "a1t", "a2t", "a3t")}

    # acc cols: 0:4 sum(bce) per group, 4:8 sum(pt), 8:12 focal sum,
    # 12:16 sum((t-0.5)*pt)
    acc_d = nc.dram_tensor("acc", [P, 20], f32, kind="ExternalOutput").ap()
    accb_d = nc.dram_tensor("accb", [P, 8], f32, kind="ExternalOutput").ap()
    acca_d = nc.dram_tensor("acca", [48, 4], f32, kind="ExternalOutput").ap()
    dice_d = nc.dram_tensor("dice", [P, P], f32, kind="ExternalOutput").ap()

    with tile.TileContext(nc) as tc:
        with (
            tc.tile_pool(name="io", bufs=3) as io,
            tc.tile_pool(name="cn", bufs=1) as cn,
            tc.tile_pool(name="ew", bufs=2) as ew,
            tc.tile_pool(name="jk", bufs=1) as jk,
            tc.tile_pool(name="ax", bufs=1) as ax,
            tc.tile_pool(name="psw", bufs=1, space="PSUM") as psw,
            tc.tile_pool(name="psd", bufs=1, space="PSUM") as psd,
            tc.tile_pool(name="psa", bufs=1, space="PSUM") as psa,
        ):
            p1 = cn.tile([P, 2, P], fp8, tag="p1")
            p2 = cn.tile([P, 2, P], fp8, tag="p2")
            p3 = cn.tile([P, 2, P], fp8, tag="p3")
            auxw = {}
            for k in ("a1", "a2", "a3", "a1t", "a2t", "a3t"):
                auxw[k] = cn.tile([96, 2, 48], fp8, tag=k, name=k)
                nc.sync.dma_start(auxw[k][:], aux_d[k][:])
            for tl, dd in ((p1, p1_d), (p2, p2_d), (p3, p3_d)):
                nc.sync.dma_start(tl[:], dd[:])

            # acc cols per chunk c (6 chunks): bce c, pt 6+c, focal 12+c
            acc = cn.tile([P, 20], f32, tag="acc")
            accb = cn.tile([P, 8], f32, tag="accb")
            acca = cn.tile([48, 4], f32, tag="acca")
            nc.vector.memset(acc[:], 0.0)
            nc.vector.memset(accb[:], 0.0)
            nc.vector.memset(acca[:], 0.0)

            dice_ps = psd.tile([P, P], f32, tag="dice")

            # aux inputs loaded early so the tail compute never waits on DMA
            auxt = []
            for h in range(2):
                mctx = ax.tile([96, W + 4], fp8, tag=f"mctx{h}")
                xw = ax.tile([48, W], bf16, tag=f"xw{h}")
                tw = ax.tile([48, W], bf16, tag=f"tw{h}")
                nc.sync.dma_start(mctx[:], mctx_d[h][:])
                nc.sync.dma_start(xw[:], xwr_d[h][:])
                nc.sync.dma_start(tw[:], twr_d[h][:])
                auxt.append((mctx, xw, tw))

            # chunks: (slot0, nslots); fine-grained at both ends to cut
            # pipeline fill/drain
            CH = [(0, 4), (4, 4), (8, 8), (16, 8), (24, 4), (28, 4)]
            NCH = len(CH)
            tiles = {}

            def conv_rhs(ms, s, c0, step):
                base = ms[:, s, :]
                pd = list(base.ap[0])
                return AP(base.tensor, base.offset + c0,
                          [pd, [step, 2], [1, W]])

            def dma(c):
                s0, ns = CH[c]
                xs = io.tile([P, ns, W], bf16, tag=f"xs{ns}", name=f"xs{c}")
                ts2 = io.tile([P, ns, W], bf16, tag=f"ts{ns}", name=f"ts{c}")
                ms = io.tile([P, ns, W + 4], fp8, tag=f"ms{ns}",
                             name=f"ms{c}")
                h = ns // 2
                nc.sync.dma_start(xs[:, 0:h, :], x_d[:, s0:s0 + h, :])
                nc.sync.dma_start(xs[:, h:ns, :], x_d[:, s0 + h:s0 + ns, :])
                nc.sync.dma_start(ts2[:, 0:h, :], t2_d[:, s0:s0 + h, :])
                nc.sync.dma_start(ts2[:, h:ns, :],
                                  t2_d[:, s0 + h:s0 + ns, :])
                nc.sync.dma_start(ms[:], m_d[:, s0:s0 + ns, :])
                tiles[c] = [xs, ts2, ms]

            def zh_op(c):
                s0, ns = CH[c]
                xs, ts2, ms = tiles[c]
                zh = ew.tile([P, ns, W], bf16, tag=f"zh{ns}", name=f"zh{c}")
                nc.vector.tensor_tensor(out=zh[:], in0=ts2[:], in1=xs[:],
                                        op=Alu.mult)
                tiles[c].append(zh)

            def acts(c):
                s0, ns = CH[c]
                xs, ts2, ms, zh = tiles[c]
                # q overwrites xs (dead after zh) to save SBUF
                q = xs
                nc.scalar.activation(q[:], zh[:], Act.Exp, scale=-0.0625)
                nb = ew.tile([P, ns, W], bf16, tag=f"nb{ns}", name=f"nb{c}")
                nc.scalar.activation(nb[:], q[:], Act.Ln, bias=1.0,
                                     accum_out=acc[:, c:c + 1])
                pt = ew.tile([P, ns, W], bf16, tag=f"pt{ns}", name=f"pt{c}")
                nc.scalar.activation(pt[:], nb[:], Act.Exp, scale=-1.0,
                                     accum_out=acc[:, 6 + c:7 + c])
                tiles[c] += [nb, pt]

            def conv(c, sb_base):
                s0, ns = CH[c]
                ms = tiles[c][2]
                for b in range(ns // 4):
                    wt = psw.tile([P, 4, W], f32, tag="wt",
                                  name=f"W_c{c}_b{b}")
                    for wi, (pw, c0, st) in enumerate(
                            ((p1, 1, 1), (p2, 0, 3), (p3, 0, 4))):
                        for i in range(4):
                            s = b * 4 + i
                            nc.tensor.matmul(wt[:, i, :], pw[:],
                                             conv_rhs(ms, s, c0, st),
                                             start=(wi == 0),
                                             stop=(wi == 2),
                                             perf_mode=DR)
                    tiles[c].append((wt, b, sb_base + b))

            def focal(c):
                s0, ns = CH[c]
                nb, pt = tiles[c][4], tiles[c][5]
                junk2 = jk.tile([P, ns, W], bf16, tag=f"junk2{ns}")
                nc.vector._custom_dve(
                    FOCAL, out=junk2[:], in0=pt[:], in1=nb[:],
                    accum_out=acc[:, 12 + c:13 + c])

            def bound(c):
                s0, ns = CH[c]
                nb = tiles[c][4]
                junk = jk.tile([P, 4, W], bf16, tag="junk")
                for wt, b, col in tiles[c][6:]:
                    nc.vector._custom_dve(
                        BOUND, out=junk[:],
                        in0=wt[:], in1=nb[:, b * 4:(b + 1) * 4, :],
                        s0=24.0, accum_out=accb[:, col:col + 1])

            def diag(c):
                s0, ns = CH[c]
                ts2, pt = tiles[c][1], tiles[c][5]
                for s in range(ns):
                    for k in range(4):
                        nc.tensor.matmul(
                            dice_ps[:],
                            ts2[:, s, k * P:(k + 1) * P],
                            pt[:, s, k * P:(k + 1) * P],
                            start=(c == 0 and s == 0 and k == 0),
                            stop=(c == NCH - 1 and s == ns - 1 and k == 3))

            def aux():
                for h in range(2):
                    mctx, xw, tw = auxt[h]
                    zw = ax.tile([48, W], bf16, tag="zw")
                    nc.vector.scalar_tensor_tensor(
                        out=zw[:], in0=tw[:], scalar=0.5, in1=xw[:],
                        op0=Alu.subtract, op1=Alu.mult)
                    qw = ax.tile([48, W], bf16, tag="qw")
                    bw = ax.tile([48, W], bf16, tag="bw")
                    nc.scalar.activation(qw[:], zw[:], Act.Exp, scale=-2.0)
                    nc.scalar.activation(bw[:], qw[:], Act.Ln, bias=1.0)
                    jw = ax.tile([48, W], bf16, tag="jw")
                    base = mctx[:, :]
                    pd = list(base.ap[0])
                    for op, col, mats in (
                        (BOUND, h, ("a1", "a2", "a3")),
                        (NBOUND, 2 + h, ("a1t", "a2t", "a3t")),
                    ):
                        wt = psa.tile([48, W], f32, tag="wtrue")
                        for wi, (wk, c0, st) in enumerate(
                                ((mats[0], 1, 1), (mats[1], 0, 3),
                                 (mats[2], 0, 4))):
                            rhs = AP(base.tensor, base.offset + c0,
                                     [pd, [st, 2], [1, W]])
                            nc.tensor.matmul(wt[:], auxw[wk][:], rhs,
                                             start=(wi == 0),
                                             stop=(wi == 2),
                                             perf_mode=DR)
                        nc.vector._custom_dve(
                            op, out=jw[:], in0=wt[:], in1=bw[:], s0=24.0,
                            accum_out=acca[:, col:col + 1])

            for c in range(NCH):
                dma(c)
            zh_op(0)
            zh_op(1)
            acts(0)
            conv(0, 0)
            zh_op(2)
            acts(1)
            conv(1, 1)
            focal(0)
            bound(0)
            acts(2)
            conv(2, 2)
            zh_op(3)
            focal(1)
            bound(1)
            diag(0)
            acts(3)
            conv(3, 4)
            focal(2)
            bound(2)
            diag(1)
            aux()
            zh_op(4)
            diag(2)
            acts(4)
            conv(4, 6)
            zh_op(5)
            focal(3)
            bound(3)
            diag(3)
            acts(5)
            conv(5, 7)
            focal(4)
            bound(4)
            diag(4)
            focal(5)
            bound(5)
            diag(5)

            nc.sync.dma_start(acc_d[:], acc[:])
            nc.sync.dma_start(accb_d[:], accb[:])
            nc.sync.dma_start(acca_d[:], acca[:])
            dsb = cn.tile([P, P], f32, tag="dsb")
            nc.scalar.copy(dsb[:], dice_ps[:])
            nc.sync.dma_start(dice_d[:], dsb[:])

    nc.compile()
    unpatch()
    return nc


def _get_nc():
    if "nc" not in _CACHE:
        _CACHE["nc"] = _build()
    return _CACHE["nc"]


def kernel(inputs: np.ndarray, targets: np.ndarray) -> np.ndarray:
    import os
    import ml_dtypes
    from concourse.bass_utils import run_bass_kernel_spmd

    bf = ml_dtypes.bfloat16
    f8 = ml_dtypes.float8_e4m3

    nc = _get_nc()
    st = _stationaries()

    x = np.asarray(inputs, dtype=np.float32).reshape(64, H, W)
    t = np.asarray(targets, dtype=np.float32).reshape(64, H, W)

    in_maps = []
    for c in range(N_CORES):
        xc = x[c * IMG:(c + 1) * IMG]     # [8, 512, 512]
        tc_ = t[c * IMG:(c + 1) * IMG]
        # transposed layout: [128 p=row-in-band, slot=img*4+band, 512]
        xT = np.ascontiguousarray(
            xc.reshape(IMG, BANDS, P, W).transpose(2, 0, 1, 3)
            .reshape(P, SLOTS, W)).astype(bf)
        tT = (tc_.reshape(IMG, BANDS, P, W).transpose(2, 0, 1, 3)
              .reshape(P, SLOTS, W))
        t2 = np.ascontiguousarray(32.0 * (tT - 0.5)).astype(bf)
        mp = np.zeros((P, SLOTS, W + 4), np.float32)
        mp[:, :, 2:2 + W] = tT
        im = {"x": xT, "t2": t2, "m": mp.astype(f8)}
        # aux: ctx rows 124+128b..131+128b, wrong rows 126+128b..129+128b
        for h in range(2):
            imgs = tc_[4 * h:4 * h + 4]
            ximgs = xc[4 * h:4 * h + 4]
            tctx = np.stack([imgs[li, 124 + 128 * b:132 + 128 * b, :]
                             for li in range(4) for b in range(3)])
            mctx = np.zeros((96, W + 4), np.float32)
            mctx[:, 2:2 + W] = tctx.reshape(96, W)
            im[f"mctx{h}"] = mctx.astype(f8)
            twr = np.stack([imgs[li, 126 + 128 * b:130 + 128 * b, :]
                            for li in range(4) for b in range(3)])
            im[f"twr{h}"] = np.ascontiguousarray(
                twr.reshape(48, W)).astype(bf)
            xwr = np.stack([ximgs[li, 126 + 128 * b:130 + 128 * b, :]
                            for li in range(4) for b in range(3)])
            im[f"xwr{h}"] = np.ascontiguousarray(
                xwr.reshape(48, W)).astype(bf)
        im.update(st)
        in_maps.append(im)

    trace = bool(os.environ.get("BASS_TRACE_KERNEL"))
    res = run_bass_kernel_spmd(nc, in_maps, core_ids=list(range(N_CORES)),
                               trace=trace)
    _CACHE["exec_time_ns"] = res.exec_time_ns

    s_bce = s_pt = s_focal = s_bnd = s_diag = 0.0
    for c in range(N_CORES):
        acc = res.results[c]["acc"].astype(np.float64)
        s_bce += acc[:, 0:6].sum()
        s_pt += acc[:, 6:12].sum()
        s_focal += acc[:, 12:18].sum()
        s_bnd += res.results[c]["accb"].astype(np.float64).sum()
        s_bnd += res.results[c]["acca"].astype(np.float64).sum()
        s_diag += np.trace(res.results[c]["dice"].astype(np.float64))

    n = float(64 * H * W)
    s_tpt = (s_diag + 16.0 * s_pt) / 32.0
    focal_loss = 0.25 * s_focal / n
    denom = n - s_pt + 2.0 * s_tpt
    dice = (2.0 * s_tpt + 1e-6) / (denom + 1e-6)
    dice_loss = 1.0 - dice
    boundary_loss = (s_bce + 5.0 * s_bnd) / n
    loss = 0.3 * focal_loss + 0.4 * dice_loss + 0.3 * boundary_loss
    return np.float32(loss)
